# revision 5
# baseline (speedup 1.0000x reference)
"""Trainium2 Bass kernel for DualTierMiras (dual low-rank tier read + LayerNorm-gate mix).

Computes, for k [N, d]:
    v_t   = k @ (SCALE * tanh(B_t @ C_t.T) + diag(D_t)).T      (t in {fast, deep})
    h     = LayerNorm(k) * gamma + beta
    w     = sigmoid(silu(h @ W1.T + b1) @ W2.T + b2 + base_logit)
    out   = w * v_fast + (1 - w) * v_deep

Strategy: data-parallel over rows across 8 NeuronCores. All device matmuls
contract over d, so every tensor is kept in a transposed layout ([d, rows]):
the host passes k.T shards and W1.T, and the device returns out.T shards.

Two device variants:
  * "lowrank": tanh(u) ~= u whenever max|u| is provably tiny (checked on the
    host with a Cauchy-Schwarz bound, and an exact max as a second resort).
    Then k @ tanh(C B^T) == (k @ C) @ B^T up to a bounded relative error and
    the tier reads are rank-32. The gate w is folded into the tiny rank-64
    intermediate, fusing both tiers into a single K=64 matmul per out tile.
  * "tanh": materializes tanh(C B^T) per 512-column block on device and does
    the full dense tier matmuls. Used when the linearization is not safe.

All matmuls run in bf16 with fp32 PSUM accumulation.
"""

from contextlib import ExitStack

import numpy as np

N, D, R = 8192, 2048, 32
NCORES = 8
NSH = N // NCORES          # rows per core
P = 128                    # SBUF partitions
NJ = D // P                # 16 chunks of d
FH = 512                   # free-dim half of NSH (PSUM bank width in fp32)
NH = NSH // FH             # 2 halves
SCALE = 0.1
LN_EPS = 1e-5
# max |B C^T| element below which tanh(u) ~= u is used (per-element relative
# error of the tanh factor <= thr^2/3 ~= 0.33%).
LOWRANK_THR = 0.10
# fp8 fast path pre-scales: k is sent as fp8 * KS, W1*gamma as fp8 * WS
# (keeps small values out of the e4m3 subnormal range), k^2 stats * QS.
KS = 16.0
WS = 64.0
QS = 2.8284271247461903

_NC_CACHE: dict = {}


# ---------------------------------------------------------------- device build

def build_nc(mode: str, has_d: bool, repeat: int = 1, sim_safe: bool = False):
    import concourse.bacc as bacc
    import concourse.tile as tile
    from concourse import mybir

    f32 = mybir.dt.float32
    nc = bacc.Bacc("TRN2", target_bir_lowering=False, debug=False,
                   num_devices=NCORES)

    bf16 = mybir.dt.bfloat16
    kt_d = nc.dram_tensor("kt", [D, NSH], bf16, kind="ExternalInput")
    w1t_d = nc.dram_tensor("w1t", [D, D], bf16, kind="ExternalInput")
    pv_d = nc.dram_tensor("pv", [P, 64], f32, kind="ExternalInput")
    sc_d = nc.dram_tensor("sc", [1, 1], f32, kind="ExternalInput")
    bt_d = nc.dram_tensor("bt", [64, D], f32, kind="ExternalInput")
    caug_d = ct_d = dv_d = None
    if mode == "lowrank":
        caug_d = nc.dram_tensor("caug", [D, 65], f32, kind="ExternalInput")
    else:
        ct_d = nc.dram_tensor("ct", [64, D], f32, kind="ExternalInput")
    if has_d:
        dv_d = nc.dram_tensor("dv", [P, 32], f32, kind="ExternalInput")
    # bf16 output (inputs to every product term are bf16 already); the host
    # upcasts to f32 when unsharding. Halves the output DMA on the tail.
    out_d = nc.dram_tensor("outT", [D, NSH], bf16, kind="ExternalOutput")

    with tile.TileContext(nc) as tc:
        for _ in range(repeat):
            with ExitStack() as ctx:
                _emit(ctx, tc, nc, mode, has_d,
                      kt_d, w1t_d, pv_d, sc_d, bt_d, caug_d, ct_d, dv_d, out_d,
                      sim_safe=sim_safe)
    nc.compile()
    return nc


def _emit(ctx, tc, nc, mode, has_d,
          kt_d, w1t_d, pv_d, sc_d, bt_d, caug_d, ct_d, dv_d, out_d,
          sim_safe=False):
    import concourse.bass as bass  # noqa: F401
    from concourse import mybir

    f32 = mybir.dt.float32
    bf16 = mybir.dt.bfloat16
    AF = mybir.ActivationFunctionType
    ALU = mybir.AluOpType
    lowrank = mode == "lowrank"

    const = ctx.enter_context(tc.tile_pool(name="const", bufs=1))
    persist = ctx.enter_context(tc.tile_pool(name="persist", bufs=1))
    stage = ctx.enter_context(tc.tile_pool(name="stage", bufs=2))
    tmp = ctx.enter_context(tc.tile_pool(name="tmp", bufs=3))
    kt2pool = ctx.enter_context(tc.tile_pool(name="kt2p", bufs=2))
    h2pool = ctx.enter_context(tc.tile_pool(name="h2p", bufs=2))
    outpool = ctx.enter_context(tc.tile_pool(name="outp", bufs=3))
    small = ctx.enter_context(tc.tile_pool(name="small", bufs=1))
    # rotating slots for short-lived [1, FH] vectors (each costs a full
    # free-dim slot across all partitions, so don't give each a unique tag)
    svec = ctx.enter_context(tc.tile_pool(name="svec", bufs=4))

    # ---- small constants -------------------------------------------------
    pv = const.tile([P, 64], f32, tag="pv", name="pv")
    nc.sync.dma_start(pv[:], pv_d[:])
    sc = const.tile([1, 1], f32, tag="sc", name="sc")
    nc.sync.dma_start(sc[:], sc_d[:])

    if lowrank:
        # one 3D-AP DMA + one cast instead of 16 tiny strided loads, so the
        # DMA queue reaches the kt tiles sooner
        caug_v = caug_d[:].rearrange("(j p) r -> p j r", p=P)
        with tc.tile_pool(name="caugstage", bufs=1) as caugstage:
            cf3 = caugstage.tile([P, NJ, 65], f32, tag="caugf", name="caugf")
            nc.sync.dma_start(cf3[:], caug_v[:])
            cb3 = const.tile([P, NJ, 65], bf16, tag="caugbf", name="caugbf")
            nc.vector.tensor_copy(cb3[:], cf3[:])
        caug_bf = [cb3[:, j, :] for j in range(NJ)]
    ones_col = const.tile([P, 1], bf16, tag="ones", name="ones")
    nc.vector.memset(ones_col[:], 1.0)
    ones_row = const.tile([1, P], bf16, tag="onesrow", name="onesrow")
    nc.vector.memset(ones_row[:], 1.0)
    # gpsimd.partition_broadcast writes garbage on HW via this compile path;
    # broadcast [1, FH] rows across partitions with a K=1 matmul instead.
    psBC = ctx.enter_context(tc.tile_pool(name="psBC", bufs=1, space="PSUM"))

    def bcast_psum(src_row_bf16, nparts):
        pb = psBC.tile([nparts, FH], f32, tag="pbc", name="pbc")
        nc.tensor.matmul(pb[:], ones_row[0:1, 0:nparts], src_row_bf16[:],
                         start=True, stop=True)
        return pb

    w2_bf = const.tile([P, NJ], bf16, tag="w2bf", name="w2bf")
    nc.vector.tensor_copy(w2_bf[:], pv[:, 48:64])

    if has_d:
        dv = const.tile([P, 32], f32, tag="dv", name="dv")
        nc.sync.dma_start(dv[:], dv_d[:])

    # ---- load k.T, cast bf16, phase-A matmuls (stats + G) ----------------
    # kt_bf is dead after hT is built (unless a tier path needs it later), so
    # it lives in its own pool that the caller scopes appropriately.
    import os
    resident_gate = (lowrank and not has_d
                     and not os.environ.get('K_NO_RESIDENT'))
    htpool = ctx.enter_context(tc.tile_pool(name="htp", bufs=1))
    ktpool = persist
    if resident_gate:
        # resident bf16 W1 j-tiles, prefetched during the prologue
        w1pool = ctx.enter_context(tc.tile_pool(name="w1p", bufs=1))
    kt_bf = [ktpool.tile([P, NSH], bf16, tag=f"ktbf{j}", name=f"ktbf{j}") for j in range(NJ)]
    mu = [small.tile([1, FH], f32, tag=f"mu{h}", name=f"mu{h}") for h in range(NH)]
    msq = [small.tile([1, FH], f32, tag=f"msq{h}", name=f"msq{h}") for h in range(NH)]
    G_sb = None
    if lowrank:
        G_sb = [persist.tile([64, FH], bf16, tag=f"gsb{h}", name=f"gsb{h}") for h in range(NH)]

    with tc.tile_pool(name="psA", bufs=1, space="PSUM") as psA:
        if lowrank:
            psum_G = [psA.tile([65, FH], f32, tag=f"psG{h}", name=f"psG{h}") for h in range(NH)]
        else:
            psum_S = [psA.tile([1, FH], f32, tag=f"psS{h}", name=f"psS{h}") for h in range(NH)]
        psum_Q = [psA.tile([1, FH], f32, tag=f"psQ{h}", name=f"psQ{h}") for h in range(NH)]

        for j in range(NJ):
            st, sp = j == 0, j == NJ - 1
            nc.sync.dma_start(kt_bf[j][:], kt_d[j * P:(j + 1) * P, :])
            kt2 = kt2pool.tile([P, NSH], bf16, tag="kt2", name="kt2")
            nc.vector.tensor_mul(kt2[:], kt_bf[j][:], kt_bf[j][:])
            for h in range(NH):
                sl = slice(h * FH, (h + 1) * FH)
                if lowrank:
                    nc.tensor.matmul(psum_G[h][:], caug_bf[j][:],
                                     kt_bf[j][:, sl], start=st, stop=sp)
                else:
                    nc.tensor.matmul(psum_S[h][:], ones_col[:],
                                     kt_bf[j][:, sl], start=st, stop=sp)
                nc.tensor.matmul(psum_Q[h][:], ones_col[:],
                                 kt2[:, sl], start=st, stop=sp)

        w1o = None
        if resident_gate:
            # per-o column blocks via 3D AP: arrival order == the gate's
            # consumption order, so o=0 can start after 1 MB instead of 8 MB.
            w1v = w1t_d[:].rearrange("(j p) o -> p j o", p=P)
            w1o = [w1pool.tile([P, NJ, P], bf16, tag=f"w1o{o}", name=f"w1o{o}")
                   for o in range(NJ)]
            for o in range(NJ):
                nc.sync.dma_start(w1o[o][:], w1v[:, :, o * P:(o + 1) * P])

        for h in range(NH):
            if lowrank:
                nc.scalar.mul(G_sb[h][:], psum_G[h][0:64, :], SCALE)
                nc.scalar.mul(mu[h][:], psum_G[h][64:65, :], 1.0 / D)
            else:
                nc.scalar.mul(mu[h][:], psum_S[h][:], 1.0 / D)
            nc.scalar.mul(msq[h][:], psum_Q[h][:], 1.0 / D)

    # factor tiles (used only by the tier reads, so loaded after kt+W1 to
    # keep them off the critical DMA prologue): joint [64, D] for the lowrank
    # K=64 fused matmul; split per-tier [32, D] tiles in tanh mode.
    if lowrank:
        bt_bf = const.tile([64, D], bf16, tag="btbf", name="btbf")
    else:
        btt_bf = [const.tile([32, D], bf16, tag=f"btbf{t}", name=f"btbf{t}")
                  for t in range(2)]
        ctt_bf = [const.tile([32, D], bf16, tag=f"ctbf{t}", name=f"ctbf{t}")
                  for t in range(2)]
    with tc.tile_pool(name="facstage", bufs=2) as facstage:
        for q in range(2):
            qs = slice(q * (D // 2), (q + 1) * (D // 2))
            if lowrank:
                btf = facstage.tile([64, D // 2], f32, tag="btf", name="btf")
                nc.sync.dma_start(btf[:], bt_d[:, qs])
                nc.vector.tensor_copy(bt_bf[:, qs], btf[:])
            else:
                for t in range(2):
                    btf = facstage.tile([32, D // 2], f32, tag="btf", name="btf")
                    nc.sync.dma_start(btf[:], bt_d[32 * t:32 * t + 32, qs])
                    nc.vector.tensor_copy(btt_bf[t][:, qs], btf[:])
                    ctf = facstage.tile([32, D // 2], f32, tag="ctf", name="ctf")
                    nc.sync.dma_start(ctf[:], ct_d[32 * t:32 * t + 32, qs])
                    nc.vector.tensor_copy(ctt_bf[t][:, qs], ctf[:])

    # ---- LN stats finalize + broadcast -----------------------------------
    mu_b = [persist.tile([P, FH], bf16, tag=f"mub{h}", name=f"mub{h}") for h in range(NH)]
    rstd_b = [persist.tile([P, FH], bf16, tag=f"rstdb{h}", name=f"rstdb{h}") for h in range(NH)]
    for h in range(NH):
        mu2 = svec.tile([1, FH], f32, tag="sv", name="mu2")
        nc.vector.tensor_mul(mu2[:], mu[h][:], mu[h][:])
        veps = svec.tile([1, FH], f32, tag="sv", name="veps")
        # (msq + eps) - mu^2
        nc.vector.scalar_tensor_tensor(veps[:], msq[h][:], LN_EPS, mu2[:],
                                       op0=ALU.add, op1=ALU.subtract)
        rinv = svec.tile([1, FH], f32, tag="sv", name="rinv")
        nc.vector.reciprocal(rinv[:], veps[:])
        rstd_bf = svec.tile([1, FH], bf16, tag="sv", name="rstd_bf")
        nc.scalar.activation(rstd_bf[:], rinv[:], AF.Sqrt)
        mu_bf = svec.tile([1, FH], bf16, tag="sv", name="mu_bf")
        nc.scalar.copy(mu_bf[:], mu[h][:])
        nc.scalar.copy(mu_b[h][:], bcast_psum(mu_bf, P)[:])
        nc.scalar.copy(rstd_b[h][:], bcast_psum(rstd_bf, P)[:])

    # ---- gate: h = LN(k)*gamma+beta; silu(h @ W1.T + b1); logit ----------
    wv = [svec.tile([1, FH], f32, tag="wvlong", bufs=2, name=f"wv{h}")
          for h in range(NH)]
    # W1.T viewed as [p, j, o] so one DMA fetches the [2048, 128] column block
    # for a given o-chunk as a [128, 16, 128] tile (partition dim = j rows).
    w1t_v = w1t_d[:].rearrange("(j p) o -> p j o", p=P)

    def emit_ln():
        ht = [htpool.tile([P, NSH], bf16, tag=f"ht{j}", name=f"ht{j}")
              for j in range(NJ)]
        for h in range(NH):           # h-major: h=0 tiles finish first
            for j in range(NJ):
                sl = slice(h * FH, (h + 1) * FH)
                t1 = tmp.tile([P, FH], bf16, tag="lnt1", name="lnt1")
                nc.vector.tensor_sub(t1[:], kt_bf[j][:, sl], mu_b[h][:])
                t2 = tmp.tile([P, FH], bf16, tag="lnt2", name="lnt2")
                nc.vector.tensor_mul(t2[:], t1[:], rstd_b[h][:])
                nc.scalar.activation(ht[j][:, sl], t2[:], AF.Identity,
                                     bias=pv[:, 16 + j:17 + j],
                                     scale=pv[:, j:j + 1])
        return ht

    def emit_silu(s1, o, h2):
        if sim_safe:
            # CoreSim has no Silu LUT; decompose (sim-only build).
            sbt = h2pool.tile([P, FH], f32, tag="sb", name="sb")
            nc.scalar.activation(sbt[:], s1[:], AF.Identity,
                                 bias=pv[:, 32 + o:33 + o])
            sig = h2pool.tile([P, FH], f32, tag="sig", name="sig")
            nc.scalar.activation(sig[:], s1[:], AF.Sigmoid,
                                 bias=pv[:, 32 + o:33 + o])
            nc.vector.tensor_mul(h2[:], sbt[:], sig[:])
        else:
            nc.scalar.activation(h2[:], s1[:], AF.Silu,
                                 bias=pv[:, 32 + o:33 + o])

    def emit_gate_col(psB, psum_L, w1b_j_aps, o, h):
        """One (o, h) gate column: 16 accumulating matmuls + silu + logit."""
        sl = slice(h * FH, (h + 1) * FH)
        s1 = psB.tile([P, FH], f32, tag="s1", name="s1")
        for j in range(NJ):
            nc.tensor.matmul(s1[:], w1b_j_aps[j], ht[j][:, sl],
                             start=(j == 0), stop=(j == NJ - 1))
        h2 = h2pool.tile([P, FH], bf16, tag="h2", name="h2")
        emit_silu(s1, o, h2)
        nc.tensor.matmul(psum_L[h][:], w2_bf[:, o:o + 1], h2[:],
                         start=(o == 0), stop=(o == NJ - 1))

    def emit_tier_lowrank(h, psC):
        """w -> Gw -> fused K=64 tier matmul -> out DMA, for one n-half."""
        wv_bf = svec.tile([1, FH], bf16, tag="sv", name="wv_bf")
        nc.vector.tensor_copy(wv_bf[:], wv[h][:])
        nb = P if has_d else 64
        pw = bcast_psum(wv_bf, nb)
        wcat = persist.tile([64, FH], bf16, tag=f"wcat{h}", name=f"wcat{h}")
        nc.scalar.copy(wcat[0:32, :], pw[0:32, :])
        nc.scalar.activation(wcat[32:64, :], pw[32:64, :], AF.Copy,
                             bias=1.0, scale=-1.0)
        if has_d:
            wb = persist.tile([P, FH], bf16, tag=f"wb128{h}", name=f"wb128{h}")
            nc.scalar.copy(wb[:], pw[:])
        Gw = persist.tile([64, FH], bf16, tag=f"gw{h}", name=f"gw{h}")
        nc.vector.tensor_mul(Gw[:], G_sb[h][:], wcat[:])
        for m in range(NJ):
            pvt = psC.tile([P, FH], f32, tag="vt", name="vt")
            nc.tensor.matmul(pvt[:], bt_bf[0:64, m * P:(m + 1) * P],
                             Gw[:], start=True, stop=True)
            ot = outpool.tile([P, FH], bf16, tag="ot", name="ot")
            if not has_d:
                # alternate engines so the psum->sbuf copies don't pile up
                if m % 2 == 0:
                    nc.scalar.copy(ot[:], pvt[:])
                else:
                    nc.vector.tensor_copy(ot[:], pvt[:])
            else:
                sl = slice(h * FH, (h + 1) * FH)
                dmix = tmp.tile([P, FH], bf16, tag="dmix", name="dmix")
                nc.vector.tensor_scalar(dmix[:], wb[:],
                                        dv[:, m:m + 1], dv[:, 16 + m:17 + m],
                                        op0=ALU.mult, op1=ALU.add)
                c = tmp.tile([P, FH], f32, tag="dc", name="dc")
                nc.vector.tensor_mul(c[:], kt_bf[m][:, sl], dmix[:])
                nc.vector.tensor_add(ot[:], pvt[:], c[:])
            nc.sync.dma_start(
                out_d[m * P:(m + 1) * P, h * FH:(h + 1) * FH], ot[:])

    if resident_gate:
        # Resident bf16 W1: one DMA+cast pass, reused by both n-halves, so
        # the gate runs h-outer and half 0's tier-read/output tail overlaps
        # half 1's gate matmuls.  kt_bf's pool closes once hT exists.
        ht = emit_ln()
        with ExitStack() as gctx:
            psC = gctx.enter_context(tc.tile_pool(name="psC", bufs=2,
                                                  space="PSUM"))
            with tc.tile_pool(name="psB", bufs=2, space="PSUM") as psB, \
                 tc.tile_pool(name="psL", bufs=1, space="PSUM") as psL:
                psum_L = [psL.tile([1, FH], f32, tag=f"psL{h}",
                                   name=f"psL{h}") for h in range(NH)]
                interleave = not os.environ.get('K_NO_INTERLEAVE')
                for h in range(NH):
                    for o in range(NJ):
                        aps = [w1o[o][:, j, :] for j in range(NJ)]
                        emit_gate_col(psB, psum_L, aps, o, h)
                    nc.scalar.activation(wv[h][:], psum_L[h][:], AF.Sigmoid,
                                         bias=sc[0:1, 0:1])
                    if interleave:
                        emit_tier_lowrank(h, psC)
                if not interleave:
                    for h in range(NH):
                        emit_tier_lowrank(h, psC)
    else:
        ht = emit_ln()
        with ExitStack() as gctx:
            w1bp = gctx.enter_context(tc.tile_pool(name="w1bp", bufs=2))
            with tc.tile_pool(name="psB", bufs=2, space="PSUM") as psB, \
                 tc.tile_pool(name="psL", bufs=1, space="PSUM") as psL:
                psum_L = [psL.tile([1, FH], f32, tag=f"psL{h}",
                                   name=f"psL{h}") for h in range(NH)]
                for o in range(NJ):
                    w1b = w1bp.tile([P, NJ, P], bf16, tag="w1b", name="w1b")
                    nc.sync.dma_start(w1b[:], w1t_v[:, :, o * P:(o + 1) * P])
                    for h in range(NH):
                        aps = [w1b[:, j, :] for j in range(NJ)]
                        emit_gate_col(psB, psum_L, aps, o, h)
                for h in range(NH):
                    nc.scalar.activation(wv[h][:], psum_L[h][:], AF.Sigmoid,
                                         bias=sc[0:1, 0:1])

    # ---- tier reads + mix ------------------------------------------------
    if lowrank:
        if not resident_gate:
            with tc.tile_pool(name="psC", bufs=3, space="PSUM") as psC:
                for h in range(NH):
                    emit_tier_lowrank(h, psC)
    else:
        # Full path: materialize M_t = tanh(C_t B_t^T) per 512-col block.
        wpb = [persist.tile([P, FH], f32, tag=f"wpb{h}", name=f"wpb{h}") for h in range(NH)]
        wqb = [persist.tile([P, FH], f32, tag=f"wqb{h}", name=f"wqb{h}") for h in range(NH)]
        wb128 = []
        for h in range(NH):
            wv_bf = svec.tile([1, FH], bf16, tag="sv", name="wv_bf")
            nc.vector.tensor_copy(wv_bf[:], wv[h][:])
            pw = bcast_psum(wv_bf, P)
            # wpb = SCALE*w, wqb = SCALE*(1-w), folded into the psum copies
            nc.scalar.mul(wpb[h][:], pw[:], SCALE)
            nc.scalar.activation(wqb[h][:], pw[:], AF.Copy,
                                 bias=SCALE, scale=-SCALE)
            if has_d:
                wb = persist.tile([P, FH], bf16, tag=f"wb128{h}", name=f"wb128{h}")
                nc.scalar.copy(wb[:], pw[:])
                wb128.append(wb)

        with ExitStack() as tctx:
            mpool = tctx.enter_context(tc.tile_pool(name="mtiles", bufs=1))
            psD = tctx.enter_context(tc.tile_pool(name="psD", bufs=2,
                                                  space="PSUM"))
            for mg in range(D // FH):
                mt = [[], []]
                for t in range(2):
                    for j in range(NJ):
                        pm = psD.tile([P, FH], f32, tag="pm", name="pm",
                                      bufs=1)
                        nc.tensor.matmul(
                            pm[:], ctt_bf[t][:, j * P:(j + 1) * P],
                            btt_bf[t][:, mg * FH:(mg + 1) * FH],
                            start=True, stop=True)
                        mtile = mpool.tile([P, FH], bf16, tag=f"m{t}_{j}", name=f"m{t}_{j}")
                        nc.scalar.activation(mtile[:], pm[:], AF.Tanh)
                        mt[t].append(mtile)
                for s in range(FH // P):
                    m = mg * (FH // P) + s
                    for h in range(NH):
                        sl = slice(h * FH, (h + 1) * FH)
                        pf = psD.tile([P, FH], f32, tag="pf", name="pf")
                        for j in range(NJ):
                            nc.tensor.matmul(pf[:],
                                             mt[0][j][:, s * P:(s + 1) * P],
                                             kt_bf[j][:, sl],
                                             start=(j == 0), stop=(j == NJ - 1))
                        pd_ = psD.tile([P, FH], f32, tag="pd", name="pd")
                        for j in range(NJ):
                            nc.tensor.matmul(pd_[:],
                                             mt[1][j][:, s * P:(s + 1) * P],
                                             kt_bf[j][:, sl],
                                             start=(j == 0), stop=(j == NJ - 1))
                        t0 = tmp.tile([P, FH], f32, tag="t0", name="t0")
                        nc.vector.tensor_mul(t0[:], pf[:], wpb[h][:])
                        t1 = tmp.tile([P, FH], f32, tag="t1", name="t1")
                        nc.vector.tensor_mul(t1[:], pd_[:], wqb[h][:])
                        ot = outpool.tile([P, FH], bf16, tag="ot", name="ot")
                        nc.vector.tensor_add(ot[:], t0[:], t1[:])
                        if has_d:
                            dmix = tmp.tile([P, FH], bf16, tag="dmix", name="dmix")
                            nc.vector.tensor_scalar(dmix[:], wb128[h][:],
                                                    dv[:, m:m + 1],
                                                    dv[:, 16 + m:17 + m],
                                                    op0=ALU.mult, op1=ALU.add)
                            c = tmp.tile([P, FH], f32, tag="dc", name="dc")
                            nc.vector.tensor_mul(c[:], kt_bf[m][:, sl], dmix[:])
                            ot2 = outpool.tile([P, FH], bf16, tag="ot2",
                                               name="ot2")
                            nc.vector.tensor_add(ot2[:], ot[:], c[:])
                            ot = ot2
                        nc.sync.dma_start(
                            out_d[m * P:(m + 1) * P, h * FH:(h + 1) * FH],
                            ot[:])


# ------------------------------------------------------- fast path (fp8 gate)

def build_nc_fast(repeat: int = 1, sim_safe: bool = False):
    """Lowrank/no-D fast path: LN folded into W1, fp8 DoubleRow gate matmul.

    Per (o, h) gate column the PSUM group is 8 fp8-DR matmuls (K=256 each)
    plus one K=1 fp16 matmul adding the rank-1 LN mean correction
    (-mu_r) * (KS*WS * W1@gamma)_o; z = psum * (rstd/(KS*WS)) then
    h2 = silu(z + (W1@beta + b1)).
    """
    import concourse.bacc as bacc
    import concourse.tile as tile
    from concourse import mybir

    f32 = mybir.dt.float32
    f16 = mybir.dt.float16
    f8 = mybir.dt.float8e4
    nc = bacc.Bacc("TRN2", target_bir_lowering=False, debug=False,
                   num_devices=NCORES)

    kt16_d = nc.dram_tensor("kt16", [D, NSH], f16, kind="ExternalInput")
    kt8_d = nc.dram_tensor("kt8", [D, NSH], f8, kind="ExternalInput")
    w1p8_d = nc.dram_tensor("w1p8", [D, D], f8, kind="ExternalInput")
    caug_d = nc.dram_tensor("caug2", [P, NJ * 65], f16, kind="ExternalInput")
    bt_d = nc.dram_tensor("bt2", [64, D], f16, kind="ExternalInput")
    c1s_d = nc.dram_tensor("c1s", [1, D], f16, kind="ExternalInput")
    cb_d = nc.dram_tensor("cb", [P, NJ], f32, kind="ExternalInput")
    w2b_d = nc.dram_tensor("w2b", [P, NJ], f16, kind="ExternalInput")
    sc_d = nc.dram_tensor("sc", [1, 1], f32, kind="ExternalInput")
    out_d = nc.dram_tensor("outF", [NH * NJ * P, FH], f16,
                           kind="ExternalOutput")

    with tile.TileContext(nc) as tc:
        for _ in range(repeat):
            with ExitStack() as ctx:
                _emit_fast(ctx, tc, nc, kt16_d, kt8_d, w1p8_d, caug_d, bt_d,
                           c1s_d, cb_d, w2b_d, sc_d, out_d, sim_safe=sim_safe)
    nc.compile()
    return nc


def _emit_fast(ctx, tc, nc, kt16_d, kt8_d, w1p8_d, caug_d, bt_d,
               c1s_d, cb_d, w2b_d, sc_d, out_d, sim_safe=False):
    from concourse import mybir

    f32 = mybir.dt.float32
    f16 = mybir.dt.float16
    f8 = mybir.dt.float8e4
    AF = mybir.ActivationFunctionType
    ALU = mybir.AluOpType
    DR = mybir.MatmulPerfMode.DoubleRow

    const = ctx.enter_context(tc.tile_pool(name="const", bufs=1))
    persist = ctx.enter_context(tc.tile_pool(name="persist", bufs=1))
    tmp = ctx.enter_context(tc.tile_pool(name="tmp", bufs=3))
    h2pool = ctx.enter_context(tc.tile_pool(name="h2p", bufs=2))
    outpool = ctx.enter_context(tc.tile_pool(name="outp", bufs=4))
    small = ctx.enter_context(tc.tile_pool(name="small", bufs=1))
    svec = ctx.enter_context(tc.tile_pool(name="svec", bufs=4))

    # ---- tiny constants --------------------------------------------------
    caug_sb = const.tile([P, NJ * 65], f16, tag="caug", name="caug")
    nc.sync.dma_start(caug_sb[:], caug_d[:])
    c1s_sb = const.tile([1, D], f16, tag="c1s", name="c1s")
    nc.sync.dma_start(c1s_sb[:], c1s_d[:])
    cb_sb = const.tile([P, NJ], f32, tag="cb", name="cb")
    nc.sync.dma_start(cb_sb[:], cb_d[:])
    w2b_sb = const.tile([P, NJ], f16, tag="w2b", name="w2b")
    nc.sync.dma_start(w2b_sb[:], w2b_d[:])
    sc = const.tile([1, 1], f32, tag="sc", name="sc")
    nc.sync.dma_start(sc[:], sc_d[:])

    ones_row = const.tile([1, P], f16, tag="onesrow", name="onesrow")
    nc.vector.memset(ones_row[:], 1.0)
    ones8 = const.tile([P, 2, 16], f8, tag="ones8", name="ones8")
    nc.vector.memset(ones8[:], 1.0)

    # ---- big input DMAs (order = criticality) ----------------------------
    kt16 = [persist.tile([P, NSH], f16, tag=f"kt16_{j}", name=f"kt16_{j}")
            for j in range(NJ)]
    for j in range(NJ):
        nc.sync.dma_start(kt16[j][:], kt16_d[j * P:(j + 1) * P, :])
    kt8 = persist.tile([P, NJ, NSH], f8, tag="kt8", name="kt8")
    nc.sync.dma_start(kt8[:], kt8_d[:].rearrange("(j p) n -> p j n", p=P))
    w1o = [persist.tile([P, NJ, P], f8, tag=f"w1o{o}", name=f"w1o{o}")
           for o in range(NJ)]
    for o in range(NJ):
        nc.sync.dma_start(w1o[o][:], w1p8_d[o * P:(o + 1) * P, :]
                          .rearrange("p (j m) -> p j m", j=NJ))
    bt_sb = const.tile([64, D], f16, tag="bt", name="bt")
    nc.sync.dma_start(bt_sb[:], bt_d[:])

    # ---- phase A: stats --------------------------------------------------
    kt28 = persist.tile([P, NJ, NSH], f8, tag="kt28", name="kt28")
    G_sb = [persist.tile([64, FH], f16, tag=f"gsb{h}", name=f"gsb{h}")
            for h in range(NH)]
    negmu = [small.tile([1, FH], f16, tag=f"negmu{h}", name=f"negmu{h}")
             for h in range(NH)]
    rstd_b = [persist.tile([P, FH], f16, tag=f"rstdb{h}", name=f"rstdb{h}")
              for h in range(NH)]
    psBC = ctx.enter_context(tc.tile_pool(name="psBC", bufs=1, space="PSUM"))

    with tc.tile_pool(name="psA", bufs=1, space="PSUM") as psA:
        psum_G = [psA.tile([65, FH], f32, tag=f"psG{h}", name=f"psG{h}")
                  for h in range(NH)]
        psum_Q = [psA.tile([1, FH], f32, tag=f"psQ{h}", name=f"psQ{h}")
                  for h in range(NH)]
        for j in range(NJ):
            st, sp = j == 0, j == NJ - 1
            nc.vector.scalar_tensor_tensor(
                kt28[:, j, :], kt16[j][:], QS, kt16[j][:],
                op0=ALU.mult, op1=ALU.mult)
            for h in range(NH):
                sl = slice(h * FH, (h + 1) * FH)
                nc.tensor.matmul(psum_G[h][:],
                                 caug_sb[:, j * 65:(j + 1) * 65],
                                 kt16[j][:, sl], start=st, stop=sp)
            if j % 2 == 1:
                jp = j // 2
                for h in range(NH):
                    sl = slice(h * FH, (h + 1) * FH)
                    nc.tensor.matmul(psum_Q[h][:], ones8[:, :, 0:1],
                                     kt28[:, 2 * jp:2 * jp + 2, sl],
                                     start=(jp == 0), stop=(jp == NJ // 2 - 1),
                                     perf_mode=DR)

        for h in range(NH):
            nc.scalar.mul(G_sb[h][:], psum_G[h][0:64, :], SCALE)
            nc.scalar.mul(negmu[h][:], psum_G[h][64:65, :], -1.0 / D)
            mu = svec.tile([1, FH], f32, tag="sv", name="mu")
            nc.scalar.mul(mu[:], psum_G[h][64:65, :], 1.0 / D)
            msq = svec.tile([1, FH], f32, tag="sv", name="msq")
            nc.scalar.mul(msq[:], psum_Q[h][:], 1.0 / (QS * D))
            mu2 = svec.tile([1, FH], f32, tag="sv", name="mu2")
            nc.vector.tensor_mul(mu2[:], mu[:], mu[:])
            veps = svec.tile([1, FH], f32, tag="sv", name="veps")
            nc.vector.scalar_tensor_tensor(veps[:], msq[:], LN_EPS, mu2[:],
                                           op0=ALU.add, op1=ALU.subtract)
            rinv = svec.tile([1, FH], f32, tag="sv", name="rinv")
            nc.vector.reciprocal(rinv[:], veps[:])
            rstd_row = svec.tile([1, FH], f16, tag="sv", name="rstd_row")
            # sqrt(rinv / (KS*WS)^2) = rstd / (KS*WS)
            nc.scalar.activation(rstd_row[:], rinv[:], AF.Sqrt,
                                 scale=1.0 / (KS * WS) ** 2)
            pb = psBC.tile([P, FH], f32, tag="pbc", name="pbc")
            nc.tensor.matmul(pb[:], ones_row[0:1, 0:P], rstd_row[:],
                             start=True, stop=True)
            nc.scalar.copy(rstd_b[h][:], pb[:])

    # ---- gate + tier, per n-half ----------------------------------------
    def emit_silu(t, o, h2):
        if sim_safe:
            sbt = h2pool.tile([P, FH], f32, tag="sb", name="sb")
            nc.scalar.activation(sbt[:], t[:], AF.Identity,
                                 bias=cb_sb[:, o:o + 1])
            sig = h2pool.tile([P, FH], f32, tag="sig", name="sig")
            nc.scalar.activation(sig[:], t[:], AF.Sigmoid,
                                 bias=cb_sb[:, o:o + 1])
            nc.vector.tensor_mul(h2[:], sbt[:], sig[:])
        else:
            nc.scalar.activation(h2[:], t[:], AF.Silu,
                                 bias=cb_sb[:, o:o + 1])

    psC = ctx.enter_context(tc.tile_pool(name="psC", bufs=3, space="PSUM"))
    with tc.tile_pool(name="psB", bufs=2, space="PSUM") as psB, \
         tc.tile_pool(name="psL", bufs=1, space="PSUM") as psL:
        psum_L = [psL.tile([1, FH], f32, tag=f"psL{h}", name=f"psL{h}")
                  for h in range(NH)]
        for h in range(NH):
            sl = slice(h * FH, (h + 1) * FH)
            for o in range(NJ):
                s1 = psB.tile([P, FH], f32, tag="s1", name="s1")
                for jp in range(NJ // 2):
                    nc.tensor.matmul(s1[:], w1o[o][:, 2 * jp:2 * jp + 2, :],
                                     kt8[:, 2 * jp:2 * jp + 2, sl],
                                     start=(jp == 0), stop=False,
                                     perf_mode=DR)
                nc.tensor.matmul(s1[:], c1s_sb[0:1, o * P:(o + 1) * P],
                                 negmu[h][:], start=False, stop=True)
                t = tmp.tile([P, FH], f16, tag="t", name="t")
                nc.vector.tensor_mul(t[:], s1[:], rstd_b[h][:])
                h2 = h2pool.tile([P, FH], f16, tag="h2", name="h2")
                emit_silu(t, o, h2)
                nc.tensor.matmul(psum_L[h][:], w2b_sb[:, o:o + 1], h2[:],
                                 start=(o == 0), stop=(o == NJ - 1))
            wv16 = svec.tile([1, FH], f16, tag="sv", name="wv16")
            nc.scalar.activation(wv16[:], psum_L[h][:], AF.Sigmoid,
                                 bias=sc[0:1, 0:1])
            pw = psBC.tile([P, FH], f32, tag="pbc", name="pw")
            nc.tensor.matmul(pw[0:64, :], ones_row[0:1, 0:64], wv16[:],
                             start=True, stop=True)
            wcat = persist.tile([64, FH], f16, tag=f"wcat{h}",
                                name=f"wcat{h}")
            nc.scalar.copy(wcat[0:32, :], pw[0:32, :])
            nc.scalar.activation(wcat[32:64, :], pw[32:64, :], AF.Copy,
                                 bias=1.0, scale=-1.0)
            Gw = persist.tile([64, FH], f16, tag=f"gw{h}", name=f"gw{h}")
            nc.vector.tensor_mul(Gw[:], G_sb[h][:], wcat[:])
            for m in range(NJ):
                pvt = psC.tile([P, FH], f32, tag="vt", name="vt")
                nc.tensor.matmul(pvt[:], bt_sb[0:64, m * P:(m + 1) * P],
                                 Gw[:], start=True, stop=True)
                ot = outpool.tile([P, FH], f16, tag="ot", name="ot")
                if m % 2 == 0:
                    nc.scalar.copy(ot[:], pvt[:])
                else:
                    nc.vector.tensor_copy(ot[:], pvt[:])
                nc.sync.dma_start(
                    out_d[(h * NJ + m) * P:(h * NJ + m + 1) * P, :], ot[:])


# ---------------------------------------------------------------- host side

def _chunked(vec):
    """[2048] -> [128, 16]; column j holds elements j*128 .. j*128+127."""
    return np.ascontiguousarray(np.asarray(vec, np.float32).reshape(NJ, P).T)


def _pick_mode(fast_B, fast_C, deep_B, deep_C):
    """lowrank iff max |B C^T| provably <= LOWRANK_THR."""
    worst = 0.0
    for B, C in ((fast_B, fast_C), (deep_B, deep_C)):
        bound = (np.linalg.norm(B, axis=1).max() *
                 np.linalg.norm(C, axis=1).max())
        if bound > LOWRANK_THR:
            bound = float(np.abs(B @ C.T).max())
        worst = max(worst, float(bound))
    return "lowrank" if worst <= LOWRANK_THR else "tanh"


def prepare_fast(g):
    """in_maps for the fp8 fast path (lowrank, no diag-D)."""
    import ml_dtypes
    f8 = ml_dtypes.float8_e4m3
    f16 = np.float16
    k = g["k"]
    W1g = g["gate_W1"] * g["ln_gamma"][None, :]
    w1p8 = np.ascontiguousarray(
        (W1g * WS).reshape(NJ, P, NJ, P).transpose(0, 3, 2, 1)
        .reshape(D, D)).astype(f8)
    caug = np.concatenate([g["fast_C"], g["deep_C"],
                           np.ones((D, 1), np.float32)], axis=1)
    caug2 = np.ascontiguousarray(
        caug.reshape(NJ, P, 65).transpose(1, 0, 2).reshape(P, NJ * 65)
    ).astype(f16)
    common = {
        "w1p8": w1p8,
        "caug2": caug2,
        "bt2": np.ascontiguousarray(
            np.concatenate([g["fast_B"].T, g["deep_B"].T], axis=0)
        ).astype(f16),
        "c1s": ((g["gate_W1"] @ g["ln_gamma"]) * (KS * WS)
                ).astype(f16).reshape(1, D),
        "cb": _chunked(g["gate_W1"] @ g["ln_beta"] + g["gate_b1"]),
        "w2b": _chunked(g["gate_W2"][0]).astype(f16),
        "sc": np.array([[g["gate_b2"][0] + g["base_logit"][0]]], np.float32),
    }
    in_maps = []
    for i in range(NCORES):
        m = dict(common)
        ktT = np.ascontiguousarray(k[i * NSH:(i + 1) * NSH, :].T)
        m["kt16"] = ktT.astype(f16)
        m["kt8"] = (ktT * KS).astype(f8)
        in_maps.append(m)
    return in_maps


def prepare(inputs):
    """-> (mode, has_d, in_maps) for the 8 cores."""
    g = {k: np.asarray(v, np.float32) for k, v in inputs.items()}
    k = g["k"]
    assert k.shape == (N, D), k.shape

    mode = _pick_mode(g["fast_B"], g["fast_C"], g["deep_B"], g["deep_C"])
    has_d = bool(np.any(g["fast_D"]) or np.any(g["deep_D"]))
    if mode == "lowrank" and not has_d:
        return "fast", False, prepare_fast(g)

    pv = np.concatenate([_chunked(g["ln_gamma"]), _chunked(g["ln_beta"]),
                         _chunked(g["gate_b1"]), _chunked(g["gate_W2"][0])],
                        axis=1)
    import ml_dtypes
    bf = ml_dtypes.bfloat16
    common = {
        "w1t": np.ascontiguousarray(g["gate_W1"].T).astype(bf),
        "pv": pv,
        "sc": np.array([[g["gate_b2"][0] + g["base_logit"][0]]], np.float32),
        "bt": np.ascontiguousarray(
            np.concatenate([g["fast_B"].T, g["deep_B"].T], axis=0)),
    }
    if mode == "lowrank":
        common["caug"] = np.ascontiguousarray(
            np.concatenate([g["fast_C"], g["deep_C"],
                            np.ones((D, 1), np.float32)], axis=1))
    else:
        common["ct"] = np.ascontiguousarray(
            np.concatenate([g["fast_C"].T, g["deep_C"].T], axis=0))
    if has_d:
        common["dv"] = np.ascontiguousarray(
            np.concatenate([_chunked(g["fast_D"] - g["deep_D"]),
                            _chunked(g["deep_D"])], axis=1))

    in_maps = []
    for i in range(NCORES):
        m = dict(common)
        m["kt"] = np.ascontiguousarray(
            k[i * NSH:(i + 1) * NSH, :].T).astype(bf)
        in_maps.append(m)
    return mode, has_d, in_maps


def get_nc(mode, has_d, repeat=1, sim_safe=False):
    key = (mode, has_d, repeat, sim_safe)
    if key not in _NC_CACHE:
        if mode == "fast":
            _NC_CACHE[key] = build_nc_fast(repeat, sim_safe)
        else:
            _NC_CACHE[key] = build_nc(mode, has_d, repeat, sim_safe)
    return _NC_CACHE[key]


def unscramble_fast(outF):
    """[NH*NJ*P, FH] tile-flat f16 -> [NSH, D] f32 row-major shard."""
    return (np.asarray(outF).astype(np.float32)
            .reshape(NH, NJ, P, FH).transpose(0, 3, 1, 2).reshape(NSH, D))


def assemble(results):
    out = np.empty((N, D), np.float32)
    for i in range(NCORES):
        if "outF" in results[i]:
            out[i * NSH:(i + 1) * NSH, :] = unscramble_fast(results[i]["outF"])
        else:
            out[i * NSH:(i + 1) * NSH, :] = \
                results[i]["outT"].astype(np.float32).T
    return out


def kernel(**inputs) -> np.ndarray:
    from concourse.bass_utils import run_bass_kernel_spmd

    mode, has_d, in_maps = prepare(inputs)
    nc = get_nc(mode, has_d)
    res = run_bass_kernel_spmd(nc, in_maps, core_ids=list(range(NCORES)))
    return assemble(res.results)



# revision 15
# speedup vs baseline: 1.0282x; 1.0282x over previous
"""Trainium2 Bass kernel for DualTierMiras (dual low-rank tier read + LayerNorm-gate mix).

Computes, for k [N, d]:
    v_t   = k @ (SCALE * tanh(B_t @ C_t.T) + diag(D_t)).T      (t in {fast, deep})
    h     = LayerNorm(k) * gamma + beta
    w     = sigmoid(silu(h @ W1.T + b1) @ W2.T + b2 + base_logit)
    out   = w * v_fast + (1 - w) * v_deep

Strategy: data-parallel over rows across 8 NeuronCores. All device matmuls
contract over d, so every tensor is kept in a transposed layout ([d, rows]):
the host passes k.T shards and W1.T, and the device returns out.T shards.

Two device variants:
  * "lowrank": tanh(u) ~= u whenever max|u| is provably tiny (checked on the
    host with a Cauchy-Schwarz bound, and an exact max as a second resort).
    Then k @ tanh(C B^T) == (k @ C) @ B^T up to a bounded relative error and
    the tier reads are rank-32. The gate w is folded into the tiny rank-64
    intermediate, fusing both tiers into a single K=64 matmul per out tile.
  * "tanh": materializes tanh(C B^T) per 512-column block on device and does
    the full dense tier matmuls. Used when the linearization is not safe.

All matmuls run in bf16 with fp32 PSUM accumulation.
"""

from contextlib import ExitStack

import numpy as np

N, D, R = 8192, 2048, 32
NCORES = 8
NSH = N // NCORES          # rows per core
P = 128                    # SBUF partitions
NJ = D // P                # 16 chunks of d
FH = 512                   # free-dim half of NSH (PSUM bank width in fp32)
NH = NSH // FH             # 2 halves
SCALE = 0.1
LN_EPS = 1e-5
# max |B C^T| element below which tanh(u) ~= u is used (per-element relative
# error of the tanh factor <= thr^2/3 ~= 0.33%).
LOWRANK_THR = 0.10
# fp8 fast path pre-scales: k is sent as fp8 * KS, W1*gamma as fp8 * WS
# (keeps small values out of the e4m3 subnormal range), k^2 stats * QS.
KS = 16.0
WS = 64.0
QS = 2.8284271247461903

_NC_CACHE: dict = {}


# ---------------------------------------------------------------- device build

def build_nc(mode: str, has_d: bool, repeat: int = 1, sim_safe: bool = False):
    import concourse.bacc as bacc
    import concourse.tile as tile
    from concourse import mybir

    f32 = mybir.dt.float32
    nc = bacc.Bacc("TRN2", target_bir_lowering=False, debug=False,
                   num_devices=NCORES)

    bf16 = mybir.dt.bfloat16
    kt_d = nc.dram_tensor("kt", [D, NSH], bf16, kind="ExternalInput")
    w1t_d = nc.dram_tensor("w1t", [D, D], bf16, kind="ExternalInput")
    pv_d = nc.dram_tensor("pv", [P, 64], f32, kind="ExternalInput")
    sc_d = nc.dram_tensor("sc", [1, 1], f32, kind="ExternalInput")
    bt_d = nc.dram_tensor("bt", [64, D], f32, kind="ExternalInput")
    caug_d = ct_d = dv_d = None
    if mode == "lowrank":
        caug_d = nc.dram_tensor("caug", [D, 65], f32, kind="ExternalInput")
    else:
        ct_d = nc.dram_tensor("ct", [64, D], f32, kind="ExternalInput")
    if has_d:
        dv_d = nc.dram_tensor("dv", [P, 32], f32, kind="ExternalInput")
    # bf16 output (inputs to every product term are bf16 already); the host
    # upcasts to f32 when unsharding. Halves the output DMA on the tail.
    out_d = nc.dram_tensor("outT", [D, NSH], bf16, kind="ExternalOutput")

    with tile.TileContext(nc) as tc:
        for _ in range(repeat):
            with ExitStack() as ctx:
                _emit(ctx, tc, nc, mode, has_d,
                      kt_d, w1t_d, pv_d, sc_d, bt_d, caug_d, ct_d, dv_d, out_d,
                      sim_safe=sim_safe)
    nc.compile()
    return nc


def _emit(ctx, tc, nc, mode, has_d,
          kt_d, w1t_d, pv_d, sc_d, bt_d, caug_d, ct_d, dv_d, out_d,
          sim_safe=False):
    import concourse.bass as bass  # noqa: F401
    from concourse import mybir

    f32 = mybir.dt.float32
    bf16 = mybir.dt.bfloat16
    AF = mybir.ActivationFunctionType
    ALU = mybir.AluOpType
    lowrank = mode == "lowrank"

    const = ctx.enter_context(tc.tile_pool(name="const", bufs=1))
    persist = ctx.enter_context(tc.tile_pool(name="persist", bufs=1))
    stage = ctx.enter_context(tc.tile_pool(name="stage", bufs=2))
    tmp = ctx.enter_context(tc.tile_pool(name="tmp", bufs=3))
    kt2pool = ctx.enter_context(tc.tile_pool(name="kt2p", bufs=2))
    h2pool = ctx.enter_context(tc.tile_pool(name="h2p", bufs=2))
    outpool = ctx.enter_context(tc.tile_pool(name="outp", bufs=3))
    small = ctx.enter_context(tc.tile_pool(name="small", bufs=1))
    # rotating slots for short-lived [1, FH] vectors (each costs a full
    # free-dim slot across all partitions, so don't give each a unique tag)
    svec = ctx.enter_context(tc.tile_pool(name="svec", bufs=4))

    # ---- small constants -------------------------------------------------
    pv = const.tile([P, 64], f32, tag="pv", name="pv")
    nc.sync.dma_start(pv[:], pv_d[:])
    sc = const.tile([1, 1], f32, tag="sc", name="sc")
    nc.sync.dma_start(sc[:], sc_d[:])

    if lowrank:
        # one 3D-AP DMA + one cast instead of 16 tiny strided loads, so the
        # DMA queue reaches the kt tiles sooner
        caug_v = caug_d[:].rearrange("(j p) r -> p j r", p=P)
        with tc.tile_pool(name="caugstage", bufs=1) as caugstage:
            cf3 = caugstage.tile([P, NJ, 65], f32, tag="caugf", name="caugf")
            nc.sync.dma_start(cf3[:], caug_v[:])
            cb3 = const.tile([P, NJ, 65], bf16, tag="caugbf", name="caugbf")
            nc.vector.tensor_copy(cb3[:], cf3[:])
        caug_bf = [cb3[:, j, :] for j in range(NJ)]
    ones_col = const.tile([P, 1], bf16, tag="ones", name="ones")
    nc.vector.memset(ones_col[:], 1.0)
    ones_row = const.tile([1, P], bf16, tag="onesrow", name="onesrow")
    nc.vector.memset(ones_row[:], 1.0)
    # gpsimd.partition_broadcast writes garbage on HW via this compile path;
    # broadcast [1, FH] rows across partitions with a K=1 matmul instead.
    psBC = ctx.enter_context(tc.tile_pool(name="psBC", bufs=1, space="PSUM"))

    def bcast_psum(src_row_bf16, nparts):
        pb = psBC.tile([nparts, FH], f32, tag="pbc", name="pbc")
        nc.tensor.matmul(pb[:], ones_row[0:1, 0:nparts], src_row_bf16[:],
                         start=True, stop=True)
        return pb

    w2_bf = const.tile([P, NJ], bf16, tag="w2bf", name="w2bf")
    nc.vector.tensor_copy(w2_bf[:], pv[:, 48:64])

    if has_d:
        dv = const.tile([P, 32], f32, tag="dv", name="dv")
        nc.sync.dma_start(dv[:], dv_d[:])

    # ---- load k.T, cast bf16, phase-A matmuls (stats + G) ----------------
    # kt_bf is dead after hT is built (unless a tier path needs it later), so
    # it lives in its own pool that the caller scopes appropriately.
    import os
    resident_gate = (lowrank and not has_d
                     and not os.environ.get('K_NO_RESIDENT'))
    htpool = ctx.enter_context(tc.tile_pool(name="htp", bufs=1))
    ktpool = persist
    if resident_gate:
        # resident bf16 W1 j-tiles, prefetched during the prologue
        w1pool = ctx.enter_context(tc.tile_pool(name="w1p", bufs=1))
    kt_bf = [ktpool.tile([P, NSH], bf16, tag=f"ktbf{j}", name=f"ktbf{j}") for j in range(NJ)]
    mu = [small.tile([1, FH], f32, tag=f"mu{h}", name=f"mu{h}") for h in range(NH)]
    msq = [small.tile([1, FH], f32, tag=f"msq{h}", name=f"msq{h}") for h in range(NH)]
    G_sb = None
    if lowrank:
        G_sb = [persist.tile([64, FH], bf16, tag=f"gsb{h}", name=f"gsb{h}") for h in range(NH)]

    with tc.tile_pool(name="psA", bufs=1, space="PSUM") as psA:
        if lowrank:
            psum_G = [psA.tile([65, FH], f32, tag=f"psG{h}", name=f"psG{h}") for h in range(NH)]
        else:
            psum_S = [psA.tile([1, FH], f32, tag=f"psS{h}", name=f"psS{h}") for h in range(NH)]
        psum_Q = [psA.tile([1, FH], f32, tag=f"psQ{h}", name=f"psQ{h}") for h in range(NH)]

        for j in range(NJ):
            st, sp = j == 0, j == NJ - 1
            nc.sync.dma_start(kt_bf[j][:], kt_d[j * P:(j + 1) * P, :])
            kt2 = kt2pool.tile([P, NSH], bf16, tag="kt2", name="kt2")
            nc.vector.tensor_mul(kt2[:], kt_bf[j][:], kt_bf[j][:])
            for h in range(NH):
                sl = slice(h * FH, (h + 1) * FH)
                if lowrank:
                    nc.tensor.matmul(psum_G[h][:], caug_bf[j][:],
                                     kt_bf[j][:, sl], start=st, stop=sp)
                else:
                    nc.tensor.matmul(psum_S[h][:], ones_col[:],
                                     kt_bf[j][:, sl], start=st, stop=sp)
                nc.tensor.matmul(psum_Q[h][:], ones_col[:],
                                 kt2[:, sl], start=st, stop=sp)

        w1o = None
        if resident_gate:
            # per-o column blocks via 3D AP: arrival order == the gate's
            # consumption order, so o=0 can start after 1 MB instead of 8 MB.
            w1v = w1t_d[:].rearrange("(j p) o -> p j o", p=P)
            w1o = [w1pool.tile([P, NJ, P], bf16, tag=f"w1o{o}", name=f"w1o{o}")
                   for o in range(NJ)]
            for o in range(NJ):
                nc.sync.dma_start(w1o[o][:], w1v[:, :, o * P:(o + 1) * P])

        for h in range(NH):
            if lowrank:
                nc.scalar.mul(G_sb[h][:], psum_G[h][0:64, :], SCALE)
                nc.scalar.mul(mu[h][:], psum_G[h][64:65, :], 1.0 / D)
            else:
                nc.scalar.mul(mu[h][:], psum_S[h][:], 1.0 / D)
            nc.scalar.mul(msq[h][:], psum_Q[h][:], 1.0 / D)

    # factor tiles (used only by the tier reads, so loaded after kt+W1 to
    # keep them off the critical DMA prologue): joint [64, D] for the lowrank
    # K=64 fused matmul; split per-tier [32, D] tiles in tanh mode.
    if lowrank:
        bt_bf = const.tile([64, D], bf16, tag="btbf", name="btbf")
    else:
        btt_bf = [const.tile([32, D], bf16, tag=f"btbf{t}", name=f"btbf{t}")
                  for t in range(2)]
        ctt_bf = [const.tile([32, D], bf16, tag=f"ctbf{t}", name=f"ctbf{t}")
                  for t in range(2)]
    with tc.tile_pool(name="facstage", bufs=2) as facstage:
        for q in range(2):
            qs = slice(q * (D // 2), (q + 1) * (D // 2))
            if lowrank:
                btf = facstage.tile([64, D // 2], f32, tag="btf", name="btf")
                nc.sync.dma_start(btf[:], bt_d[:, qs])
                nc.vector.tensor_copy(bt_bf[:, qs], btf[:])
            else:
                for t in range(2):
                    btf = facstage.tile([32, D // 2], f32, tag="btf", name="btf")
                    nc.sync.dma_start(btf[:], bt_d[32 * t:32 * t + 32, qs])
                    nc.vector.tensor_copy(btt_bf[t][:, qs], btf[:])
                    ctf = facstage.tile([32, D // 2], f32, tag="ctf", name="ctf")
                    nc.sync.dma_start(ctf[:], ct_d[32 * t:32 * t + 32, qs])
                    nc.vector.tensor_copy(ctt_bf[t][:, qs], ctf[:])

    # ---- LN stats finalize + broadcast -----------------------------------
    mu_b = [persist.tile([P, FH], bf16, tag=f"mub{h}", name=f"mub{h}") for h in range(NH)]
    rstd_b = [persist.tile([P, FH], bf16, tag=f"rstdb{h}", name=f"rstdb{h}") for h in range(NH)]
    for h in range(NH):
        mu2 = svec.tile([1, FH], f32, tag="sv", name="mu2")
        nc.vector.tensor_mul(mu2[:], mu[h][:], mu[h][:])
        veps = svec.tile([1, FH], f32, tag="sv", name="veps")
        # (msq + eps) - mu^2
        nc.vector.scalar_tensor_tensor(veps[:], msq[h][:], LN_EPS, mu2[:],
                                       op0=ALU.add, op1=ALU.subtract)
        rinv = svec.tile([1, FH], f32, tag="sv", name="rinv")
        nc.vector.reciprocal(rinv[:], veps[:])
        rstd_bf = svec.tile([1, FH], bf16, tag="sv", name="rstd_bf")
        nc.scalar.activation(rstd_bf[:], rinv[:], AF.Sqrt)
        mu_bf = svec.tile([1, FH], bf16, tag="sv", name="mu_bf")
        nc.scalar.copy(mu_bf[:], mu[h][:])
        nc.scalar.copy(mu_b[h][:], bcast_psum(mu_bf, P)[:])
        nc.scalar.copy(rstd_b[h][:], bcast_psum(rstd_bf, P)[:])

    # ---- gate: h = LN(k)*gamma+beta; silu(h @ W1.T + b1); logit ----------
    wv = [svec.tile([1, FH], f32, tag="wvlong", bufs=2, name=f"wv{h}")
          for h in range(NH)]
    # W1.T viewed as [p, j, o] so one DMA fetches the [2048, 128] column block
    # for a given o-chunk as a [128, 16, 128] tile (partition dim = j rows).
    w1t_v = w1t_d[:].rearrange("(j p) o -> p j o", p=P)

    def emit_ln():
        ht = [htpool.tile([P, NSH], bf16, tag=f"ht{j}", name=f"ht{j}")
              for j in range(NJ)]
        for h in range(NH):           # h-major: h=0 tiles finish first
            for j in range(NJ):
                sl = slice(h * FH, (h + 1) * FH)
                t1 = tmp.tile([P, FH], bf16, tag="lnt1", name="lnt1")
                nc.vector.tensor_sub(t1[:], kt_bf[j][:, sl], mu_b[h][:])
                t2 = tmp.tile([P, FH], bf16, tag="lnt2", name="lnt2")
                nc.vector.tensor_mul(t2[:], t1[:], rstd_b[h][:])
                nc.scalar.activation(ht[j][:, sl], t2[:], AF.Identity,
                                     bias=pv[:, 16 + j:17 + j],
                                     scale=pv[:, j:j + 1])
        return ht

    def emit_silu(s1, o, h2):
        if sim_safe:
            # CoreSim has no Silu LUT; decompose (sim-only build).
            sbt = h2pool.tile([P, FH], f32, tag="sb", name="sb")
            nc.scalar.activation(sbt[:], s1[:], AF.Identity,
                                 bias=pv[:, 32 + o:33 + o])
            sig = h2pool.tile([P, FH], f32, tag="sig", name="sig")
            nc.scalar.activation(sig[:], s1[:], AF.Sigmoid,
                                 bias=pv[:, 32 + o:33 + o])
            nc.vector.tensor_mul(h2[:], sbt[:], sig[:])
        else:
            nc.scalar.activation(h2[:], s1[:], AF.Silu,
                                 bias=pv[:, 32 + o:33 + o])

    def emit_gate_col(psB, psum_L, w1b_j_aps, o, h):
        """One (o, h) gate column: 16 accumulating matmuls + silu + logit."""
        sl = slice(h * FH, (h + 1) * FH)
        s1 = psB.tile([P, FH], f32, tag="s1", name="s1")
        for j in range(NJ):
            nc.tensor.matmul(s1[:], w1b_j_aps[j], ht[j][:, sl],
                             start=(j == 0), stop=(j == NJ - 1))
        h2 = h2pool.tile([P, FH], bf16, tag="h2", name="h2")
        emit_silu(s1, o, h2)
        nc.tensor.matmul(psum_L[h][:], w2_bf[:, o:o + 1], h2[:],
                         start=(o == 0), stop=(o == NJ - 1))

    def emit_tier_lowrank(h, psC):
        """w -> Gw -> fused K=64 tier matmul -> out DMA, for one n-half."""
        wv_bf = svec.tile([1, FH], bf16, tag="sv", name="wv_bf")
        nc.vector.tensor_copy(wv_bf[:], wv[h][:])
        nb = P if has_d else 64
        pw = bcast_psum(wv_bf, nb)
        wcat = persist.tile([64, FH], bf16, tag=f"wcat{h}", name=f"wcat{h}")
        nc.scalar.copy(wcat[0:32, :], pw[0:32, :])
        nc.scalar.activation(wcat[32:64, :], pw[32:64, :], AF.Copy,
                             bias=1.0, scale=-1.0)
        if has_d:
            wb = persist.tile([P, FH], bf16, tag=f"wb128{h}", name=f"wb128{h}")
            nc.scalar.copy(wb[:], pw[:])
        Gw = persist.tile([64, FH], bf16, tag=f"gw{h}", name=f"gw{h}")
        nc.vector.tensor_mul(Gw[:], G_sb[h][:], wcat[:])
        for m in range(NJ):
            pvt = psC.tile([P, FH], f32, tag="vt", name="vt")
            nc.tensor.matmul(pvt[:], bt_bf[0:64, m * P:(m + 1) * P],
                             Gw[:], start=True, stop=True)
            ot = outpool.tile([P, FH], bf16, tag="ot", name="ot")
            if not has_d:
                # alternate engines so the psum->sbuf copies don't pile up
                if m % 2 == 0:
                    nc.scalar.copy(ot[:], pvt[:])
                else:
                    nc.vector.tensor_copy(ot[:], pvt[:])
            else:
                sl = slice(h * FH, (h + 1) * FH)
                dmix = tmp.tile([P, FH], bf16, tag="dmix", name="dmix")
                nc.vector.tensor_scalar(dmix[:], wb[:],
                                        dv[:, m:m + 1], dv[:, 16 + m:17 + m],
                                        op0=ALU.mult, op1=ALU.add)
                c = tmp.tile([P, FH], f32, tag="dc", name="dc")
                nc.vector.tensor_mul(c[:], kt_bf[m][:, sl], dmix[:])
                nc.vector.tensor_add(ot[:], pvt[:], c[:])
            nc.sync.dma_start(
                out_d[m * P:(m + 1) * P, h * FH:(h + 1) * FH], ot[:])

    if resident_gate:
        # Resident bf16 W1: one DMA+cast pass, reused by both n-halves, so
        # the gate runs h-outer and half 0's tier-read/output tail overlaps
        # half 1's gate matmuls.  kt_bf's pool closes once hT exists.
        ht = emit_ln()
        with ExitStack() as gctx:
            psC = gctx.enter_context(tc.tile_pool(name="psC", bufs=2,
                                                  space="PSUM"))
            with tc.tile_pool(name="psB", bufs=2, space="PSUM") as psB, \
                 tc.tile_pool(name="psL", bufs=1, space="PSUM") as psL:
                psum_L = [psL.tile([1, FH], f32, tag=f"psL{h}",
                                   name=f"psL{h}") for h in range(NH)]
                interleave = not os.environ.get('K_NO_INTERLEAVE')
                for h in range(NH):
                    for o in range(NJ):
                        aps = [w1o[o][:, j, :] for j in range(NJ)]
                        emit_gate_col(psB, psum_L, aps, o, h)
                    nc.scalar.activation(wv[h][:], psum_L[h][:], AF.Sigmoid,
                                         bias=sc[0:1, 0:1])
                    if interleave:
                        emit_tier_lowrank(h, psC)
                if not interleave:
                    for h in range(NH):
                        emit_tier_lowrank(h, psC)
    else:
        ht = emit_ln()
        with ExitStack() as gctx:
            w1bp = gctx.enter_context(tc.tile_pool(name="w1bp", bufs=2))
            with tc.tile_pool(name="psB", bufs=2, space="PSUM") as psB, \
                 tc.tile_pool(name="psL", bufs=1, space="PSUM") as psL:
                psum_L = [psL.tile([1, FH], f32, tag=f"psL{h}",
                                   name=f"psL{h}") for h in range(NH)]
                for o in range(NJ):
                    w1b = w1bp.tile([P, NJ, P], bf16, tag="w1b", name="w1b")
                    nc.sync.dma_start(w1b[:], w1t_v[:, :, o * P:(o + 1) * P])
                    for h in range(NH):
                        aps = [w1b[:, j, :] for j in range(NJ)]
                        emit_gate_col(psB, psum_L, aps, o, h)
                for h in range(NH):
                    nc.scalar.activation(wv[h][:], psum_L[h][:], AF.Sigmoid,
                                         bias=sc[0:1, 0:1])

    # ---- tier reads + mix ------------------------------------------------
    if lowrank:
        if not resident_gate:
            with tc.tile_pool(name="psC", bufs=3, space="PSUM") as psC:
                for h in range(NH):
                    emit_tier_lowrank(h, psC)
    else:
        # Full path: materialize M_t = tanh(C_t B_t^T) per 512-col block.
        wpb = [persist.tile([P, FH], f32, tag=f"wpb{h}", name=f"wpb{h}") for h in range(NH)]
        wqb = [persist.tile([P, FH], f32, tag=f"wqb{h}", name=f"wqb{h}") for h in range(NH)]
        wb128 = []
        for h in range(NH):
            wv_bf = svec.tile([1, FH], bf16, tag="sv", name="wv_bf")
            nc.vector.tensor_copy(wv_bf[:], wv[h][:])
            pw = bcast_psum(wv_bf, P)
            # wpb = SCALE*w, wqb = SCALE*(1-w), folded into the psum copies
            nc.scalar.mul(wpb[h][:], pw[:], SCALE)
            nc.scalar.activation(wqb[h][:], pw[:], AF.Copy,
                                 bias=SCALE, scale=-SCALE)
            if has_d:
                wb = persist.tile([P, FH], bf16, tag=f"wb128{h}", name=f"wb128{h}")
                nc.scalar.copy(wb[:], pw[:])
                wb128.append(wb)

        with ExitStack() as tctx:
            mpool = tctx.enter_context(tc.tile_pool(name="mtiles", bufs=1))
            psD = tctx.enter_context(tc.tile_pool(name="psD", bufs=2,
                                                  space="PSUM"))
            for mg in range(D // FH):
                mt = [[], []]
                for t in range(2):
                    for j in range(NJ):
                        pm = psD.tile([P, FH], f32, tag="pm", name="pm",
                                      bufs=1)
                        nc.tensor.matmul(
                            pm[:], ctt_bf[t][:, j * P:(j + 1) * P],
                            btt_bf[t][:, mg * FH:(mg + 1) * FH],
                            start=True, stop=True)
                        mtile = mpool.tile([P, FH], bf16, tag=f"m{t}_{j}", name=f"m{t}_{j}")
                        nc.scalar.activation(mtile[:], pm[:], AF.Tanh)
                        mt[t].append(mtile)
                for s in range(FH // P):
                    m = mg * (FH // P) + s
                    for h in range(NH):
                        sl = slice(h * FH, (h + 1) * FH)
                        pf = psD.tile([P, FH], f32, tag="pf", name="pf")
                        for j in range(NJ):
                            nc.tensor.matmul(pf[:],
                                             mt[0][j][:, s * P:(s + 1) * P],
                                             kt_bf[j][:, sl],
                                             start=(j == 0), stop=(j == NJ - 1))
                        pd_ = psD.tile([P, FH], f32, tag="pd", name="pd")
                        for j in range(NJ):
                            nc.tensor.matmul(pd_[:],
                                             mt[1][j][:, s * P:(s + 1) * P],
                                             kt_bf[j][:, sl],
                                             start=(j == 0), stop=(j == NJ - 1))
                        t0 = tmp.tile([P, FH], f32, tag="t0", name="t0")
                        nc.vector.tensor_mul(t0[:], pf[:], wpb[h][:])
                        t1 = tmp.tile([P, FH], f32, tag="t1", name="t1")
                        nc.vector.tensor_mul(t1[:], pd_[:], wqb[h][:])
                        ot = outpool.tile([P, FH], bf16, tag="ot", name="ot")
                        nc.vector.tensor_add(ot[:], t0[:], t1[:])
                        if has_d:
                            dmix = tmp.tile([P, FH], bf16, tag="dmix", name="dmix")
                            nc.vector.tensor_scalar(dmix[:], wb128[h][:],
                                                    dv[:, m:m + 1],
                                                    dv[:, 16 + m:17 + m],
                                                    op0=ALU.mult, op1=ALU.add)
                            c = tmp.tile([P, FH], f32, tag="dc", name="dc")
                            nc.vector.tensor_mul(c[:], kt_bf[m][:, sl], dmix[:])
                            ot2 = outpool.tile([P, FH], bf16, tag="ot2",
                                               name="ot2")
                            nc.vector.tensor_add(ot2[:], ot[:], c[:])
                            ot = ot2
                        nc.sync.dma_start(
                            out_d[m * P:(m + 1) * P, h * FH:(h + 1) * FH],
                            ot[:])


# ------------------------------------------------------- fast path (fp8 gate)

def build_nc_fast(repeat: int = 1, sim_safe: bool = False):
    """Lowrank/no-D fast path: LN folded into W1, fp8 DoubleRow gate matmul.

    Per (o, h) gate column the PSUM group is 8 fp8-DR matmuls (K=256 each)
    plus one K=1 fp16 matmul adding the rank-1 LN mean correction
    (-mu_r) * (KS*WS * W1@gamma)_o; z = psum * (rstd/(KS*WS)) then
    h2 = silu(z + (W1@beta + b1)).
    """
    import concourse.bacc as bacc
    import concourse.tile as tile
    from concourse import mybir

    f32 = mybir.dt.float32
    f16 = mybir.dt.float16
    f8 = mybir.dt.float8e4
    nc = bacc.Bacc("TRN2", target_bir_lowering=False, debug=False,
                   num_devices=NCORES)

    kt16_d = nc.dram_tensor("kt16", [D, NSH], f16, kind="ExternalInput")
    kt8_d = nc.dram_tensor("kt8", [D, NSH], f8, kind="ExternalInput")
    w1p8_d = nc.dram_tensor("w1p8", [D, D], f8, kind="ExternalInput")
    caug_d = nc.dram_tensor("caug2", [P, NJ * 65], f16, kind="ExternalInput")
    bt_d = nc.dram_tensor("bt2", [64, D], f16, kind="ExternalInput")
    c1s_d = nc.dram_tensor("c1s", [1, D], f16, kind="ExternalInput")
    cb_d = nc.dram_tensor("cb", [P, NJ], f32, kind="ExternalInput")
    w2b_d = nc.dram_tensor("w2b", [P, NJ], f16, kind="ExternalInput")
    sc_d = nc.dram_tensor("sc", [1, 1], f32, kind="ExternalInput")
    out_d = nc.dram_tensor("outF", [NH * NJ * P, FH], f16,
                           kind="ExternalOutput")

    with tile.TileContext(nc) as tc:
        for _ in range(repeat):
            with ExitStack() as ctx:
                _emit_fast(ctx, tc, nc, kt16_d, kt8_d, w1p8_d, caug_d, bt_d,
                           c1s_d, cb_d, w2b_d, sc_d, out_d, sim_safe=sim_safe)
    nc.compile()
    return nc


def _emit_fast(ctx, tc, nc, kt16_d, kt8_d, w1p8_d, caug_d, bt_d,
               c1s_d, cb_d, w2b_d, sc_d, out_d, sim_safe=False):
    from concourse import mybir

    f32 = mybir.dt.float32
    f16 = mybir.dt.float16
    f8 = mybir.dt.float8e4
    AF = mybir.ActivationFunctionType
    ALU = mybir.AluOpType
    DR = mybir.MatmulPerfMode.DoubleRow

    const = ctx.enter_context(tc.tile_pool(name="const", bufs=1))
    persist = ctx.enter_context(tc.tile_pool(name="persist", bufs=1))
    tmp = ctx.enter_context(tc.tile_pool(name="tmp", bufs=3))
    h2pool = ctx.enter_context(tc.tile_pool(name="h2p", bufs=2))
    outpool = ctx.enter_context(tc.tile_pool(name="outp", bufs=4))
    small = ctx.enter_context(tc.tile_pool(name="small", bufs=1))
    svec = ctx.enter_context(tc.tile_pool(name="svec", bufs=4))

    # ---- tiny constants --------------------------------------------------
    caug_sb = const.tile([P, NJ * 65], f16, tag="caug", name="caug")
    nc.sync.dma_start(caug_sb[:], caug_d[:])
    c1s_sb = const.tile([1, D], f16, tag="c1s", name="c1s")
    nc.sync.dma_start(c1s_sb[:], c1s_d[:])
    cb_sb = const.tile([P, NJ], f32, tag="cb", name="cb")
    nc.sync.dma_start(cb_sb[:], cb_d[:])
    w2b_sb = const.tile([P, NJ], f16, tag="w2b", name="w2b")
    nc.sync.dma_start(w2b_sb[:], w2b_d[:])
    sc = const.tile([1, 1], f32, tag="sc", name="sc")
    nc.sync.dma_start(sc[:], sc_d[:])

    ones_row = const.tile([1, P], f16, tag="onesrow", name="onesrow")
    nc.vector.memset(ones_row[:], 1.0)
    ones8 = const.tile([P, 2, 16], f8, tag="ones8", name="ones8")
    nc.vector.memset(ones8[:], 1.0)

    # ---- big input DMAs (order = criticality) ----------------------------
    kt16 = [persist.tile([P, NSH], f16, tag=f"kt16_{j}", name=f"kt16_{j}")
            for j in range(NJ)]
    for j in range(NJ):
        nc.sync.dma_start(kt16[j][:], kt16_d[j * P:(j + 1) * P, :])
    w1o = [persist.tile([P, NJ, P], f8, tag=f"w1o{o}", name=f"w1o{o}")
           for o in range(NJ)]

    def dma_w1(o):
        nc.sync.dma_start(w1o[o][:], w1p8_d[o * P:(o + 1) * P, :]
                          .rearrange("p (j m) -> p j m", j=NJ))

    # first two W1 column blocks before the kt8 halves so the gate can begin
    # the moment the fp8 activations land; the rest stream behind.
    dma_w1(0)
    dma_w1(1)
    kt8 = persist.tile([P, NJ, NSH], f8, tag="kt8", name="kt8")
    kt8_v = kt8_d[:].rearrange("(j p) n -> p j n", p=P)
    HJ = NJ // 2
    nc.sync.dma_start(kt8[:, 0:HJ, :], kt8_v[:, 0:HJ, :])
    nc.sync.dma_start(kt8[:, HJ:NJ, :], kt8_v[:, HJ:NJ, :])
    for o in range(2, NJ):
        dma_w1(o)
    bt_sb = const.tile([64, D], f16, tag="bt", name="bt")
    nc.sync.dma_start(bt_sb[:], bt_d[:])

    # ---- phase A: stats --------------------------------------------------
    kt28 = persist.tile([P, NJ, NSH], f8, tag="kt28", name="kt28")
    G_sb = [persist.tile([64, FH], f16, tag=f"gsb{h}", name=f"gsb{h}")
            for h in range(NH)]
    negmu = [small.tile([1, FH], f16, tag=f"negmu{h}", name=f"negmu{h}")
             for h in range(NH)]
    rstd_b = [persist.tile([P, FH], f16, tag=f"rstdb{h}", name=f"rstdb{h}")
              for h in range(NH)]
    psBC = ctx.enter_context(tc.tile_pool(name="psBC", bufs=1, space="PSUM"))

    with tc.tile_pool(name="psA", bufs=1, space="PSUM") as psA:
        psum_G = [psA.tile([65, FH], f32, tag=f"psG{h}", name=f"psG{h}")
                  for h in range(NH)]
        psum_Q = [psA.tile([1, FH], f32, tag=f"psQ{h}", name=f"psQ{h}")
                  for h in range(NH)]
        sqs = QS ** 0.5
        for j in range(NJ):
            st, sp = j == 0, j == NJ - 1
            # split the squares across Act/DVE so neither paces phase A
            # (Square is present in every activation table: no table load)
            if j % 2 == 0:
                nc.scalar.activation(kt28[:, j, :], kt16[j][:], AF.Square,
                                     scale=sqs)
            else:
                nc.vector.scalar_tensor_tensor(
                    kt28[:, j, :], kt16[j][:], QS, kt16[j][:],
                    op0=ALU.mult, op1=ALU.mult)
            for h in range(NH):
                sl = slice(h * FH, (h + 1) * FH)
                nc.tensor.matmul(psum_G[h][:],
                                 caug_sb[:, j * 65:(j + 1) * 65],
                                 kt16[j][:, sl], start=st, stop=sp)
            if j % 2 == 1:
                jp = j // 2
                for h in range(NH):
                    sl = slice(h * FH, (h + 1) * FH)
                    nc.tensor.matmul(psum_Q[h][:], ones8[:, :, 0:1],
                                     kt28[:, 2 * jp:2 * jp + 2, sl],
                                     start=(jp == 0), stop=(jp == NJ // 2 - 1),
                                     perf_mode=DR)

        for h in range(NH):
            nc.scalar.mul(G_sb[h][:], psum_G[h][0:64, :], SCALE)
            nc.scalar.mul(negmu[h][:], psum_G[h][64:65, :], -1.0 / D)
            msq = svec.tile([1, FH], f32, tag="sv", name="msq")
            nc.scalar.mul(msq[:], psum_Q[h][:], 1.0 / (QS * D))
            mu2 = svec.tile([1, FH], f32, tag="sv", name="mu2")
            nc.vector.tensor_mul(mu2[:], negmu[h][:], negmu[h][:])
            veps = svec.tile([1, FH], f32, tag="sv", name="veps")
            nc.vector.scalar_tensor_tensor(veps[:], msq[:], LN_EPS, mu2[:],
                                           op0=ALU.add, op1=ALU.subtract)
            rinv = svec.tile([1, FH], f32, tag="sv", name="rinv")
            nc.vector.reciprocal(rinv[:], veps[:])
            rstd_row = svec.tile([1, FH], f16, tag="sv", name="rstd_row")
            # sqrt(rinv / (KS*WS)^2) = rstd / (KS*WS)
            nc.scalar.activation(rstd_row[:], rinv[:], AF.Sqrt,
                                 scale=1.0 / (KS * WS) ** 2)
            pb = psBC.tile([P, FH], f32, tag="pbc", name="pbc")
            nc.tensor.matmul(pb[:], ones_row[0:1, 0:P], rstd_row[:],
                             start=True, stop=True)
            nc.scalar.copy(rstd_b[h][:], pb[:])

    # ---- gate + tier, per n-half ----------------------------------------
    def emit_silu(t, o, h2):
        if sim_safe:
            sbt = h2pool.tile([P, FH], f32, tag="sb", name="sb")
            nc.scalar.activation(sbt[:], t[:], AF.Identity,
                                 bias=cb_sb[:, o:o + 1])
            sig = h2pool.tile([P, FH], f32, tag="sig", name="sig")
            nc.scalar.activation(sig[:], t[:], AF.Sigmoid,
                                 bias=cb_sb[:, o:o + 1])
            nc.vector.tensor_mul(h2[:], sbt[:], sig[:])
        else:
            nc.scalar.activation(h2[:], t[:], AF.Silu,
                                 bias=cb_sb[:, o:o + 1])

    psC = ctx.enter_context(tc.tile_pool(name="psC", bufs=3, space="PSUM"))
    with tc.tile_pool(name="psB", bufs=3, space="PSUM") as psB, \
         tc.tile_pool(name="psL", bufs=1, space="PSUM") as psL:
        for h in range(NH):
            sl = slice(h * FH, (h + 1) * FH)
            psum_L = psL.tile([1, FH], f32, tag="psL", name=f"psL{h}")
            for o in range(NJ):
                s1 = psB.tile([P, FH], f32, tag="s1", name="s1")
                for jp in range(NJ // 2):
                    nc.tensor.matmul(s1[:], w1o[o][:, 2 * jp:2 * jp + 2, :],
                                     kt8[:, 2 * jp:2 * jp + 2, sl],
                                     start=(jp == 0), stop=False,
                                     perf_mode=DR)
                nc.tensor.matmul(s1[:], c1s_sb[0:1, o * P:(o + 1) * P],
                                 negmu[h][:], start=False, stop=True)
                t = tmp.tile([P, FH], f16, tag="t", name="t")
                nc.vector.tensor_mul(t[:], s1[:], rstd_b[h][:])
                h2 = h2pool.tile([P, FH], f16, tag="h2", name="h2")
                emit_silu(t, o, h2)
                nc.tensor.matmul(psum_L[:], w2b_sb[:, o:o + 1], h2[:],
                                 start=(o == 0), stop=(o == NJ - 1))
            # w = sigmoid(logit+2*sch) = 0.5 + 0.5*tanh(0.5*logit + sch);
            # Tanh lives in the same activation table as Silu/Copy, so the
            # gate->tier transition needs no table swap (Sigmoid would).
            wv16 = svec.tile([1, FH], f16, tag="sv", name="wv16")
            nc.scalar.activation(wv16[:], psum_L[:], AF.Tanh,
                                 bias=sc[0:1, 0:1], scale=0.5)
            pw = psBC.tile([P, FH], f32, tag="pbc", name="pw")
            nc.tensor.matmul(pw[0:64, :], ones_row[0:1, 0:64], wv16[:],
                             start=True, stop=True)
            wcat = persist.tile([64, FH], f16, tag=f"wcat{h}",
                                name=f"wcat{h}")
            nc.scalar.activation(wcat[0:32, :], pw[0:32, :], AF.Copy,
                                 bias=0.5, scale=0.5)
            nc.scalar.activation(wcat[32:64, :], pw[32:64, :], AF.Copy,
                                 bias=0.5, scale=-0.5)
            Gw = persist.tile([64, FH], f16, tag=f"gw{h}", name=f"gw{h}")
            nc.vector.tensor_mul(Gw[:], G_sb[h][:], wcat[:])
            for m in range(NJ):
                pvt = psC.tile([P, FH], f32, tag="vt", name="vt")
                nc.tensor.matmul(pvt[:], bt_sb[0:64, m * P:(m + 1) * P],
                                 Gw[:], start=True, stop=True)
                ot = outpool.tile([P, FH], f16, tag="ot", name="ot")
                if m % 2 == 0:
                    nc.scalar.copy(ot[:], pvt[:])
                else:
                    nc.vector.tensor_copy(ot[:], pvt[:])
                nc.sync.dma_start(
                    out_d[(h * NJ + m) * P:(h * NJ + m + 1) * P, :], ot[:])


# ---------------------------------------------------------------- host side

def _chunked(vec):
    """[2048] -> [128, 16]; column j holds elements j*128 .. j*128+127."""
    return np.ascontiguousarray(np.asarray(vec, np.float32).reshape(NJ, P).T)


def _pick_mode(fast_B, fast_C, deep_B, deep_C):
    """lowrank iff max |B C^T| provably <= LOWRANK_THR."""
    worst = 0.0
    for B, C in ((fast_B, fast_C), (deep_B, deep_C)):
        bound = (np.linalg.norm(B, axis=1).max() *
                 np.linalg.norm(C, axis=1).max())
        if bound > LOWRANK_THR:
            bound = float(np.abs(B @ C.T).max())
        worst = max(worst, float(bound))
    return "lowrank" if worst <= LOWRANK_THR else "tanh"


def prepare_fast(g):
    """in_maps for the fp8 fast path (lowrank, no diag-D)."""
    import ml_dtypes
    f8 = ml_dtypes.float8_e4m3
    f16 = np.float16
    k = g["k"]
    W1g = g["gate_W1"] * g["ln_gamma"][None, :]
    w1p8 = np.ascontiguousarray(
        (W1g * WS).reshape(NJ, P, NJ, P).transpose(0, 3, 2, 1)
        .reshape(D, D)).astype(f8)
    caug = np.concatenate([g["fast_C"], g["deep_C"],
                           np.ones((D, 1), np.float32)], axis=1)
    caug2 = np.ascontiguousarray(
        caug.reshape(NJ, P, 65).transpose(1, 0, 2).reshape(P, NJ * 65)
    ).astype(f16)
    common = {
        "w1p8": w1p8,
        "caug2": caug2,
        "bt2": np.ascontiguousarray(
            np.concatenate([g["fast_B"].T, g["deep_B"].T], axis=0)
        ).astype(f16),
        "c1s": ((g["gate_W1"] @ g["ln_gamma"]) * (KS * WS)
                ).astype(f16).reshape(1, D),
        "cb": _chunked(g["gate_W1"] @ g["ln_beta"] + g["gate_b1"]),
        "w2b": _chunked(g["gate_W2"][0]).astype(f16),
        # tanh-form gate: w = 0.5 + 0.5*tanh(0.5*logit + sc), sc = (b2+base)/2
        "sc": np.array([[(g["gate_b2"][0] + g["base_logit"][0]) / 2]],
                       np.float32),
    }
    in_maps = []
    for i in range(NCORES):
        m = dict(common)
        ktT = np.ascontiguousarray(k[i * NSH:(i + 1) * NSH, :].T)
        m["kt16"] = ktT.astype(f16)
        m["kt8"] = (ktT * KS).astype(f8)
        in_maps.append(m)
    return in_maps


def prepare(inputs):
    """-> (mode, has_d, in_maps) for the 8 cores."""
    g = {k: np.asarray(v, np.float32) for k, v in inputs.items()}
    k = g["k"]
    assert k.shape == (N, D), k.shape

    mode = _pick_mode(g["fast_B"], g["fast_C"], g["deep_B"], g["deep_C"])
    has_d = bool(np.any(g["fast_D"]) or np.any(g["deep_D"]))
    if mode == "lowrank" and not has_d:
        return "fast", False, prepare_fast(g)

    pv = np.concatenate([_chunked(g["ln_gamma"]), _chunked(g["ln_beta"]),
                         _chunked(g["gate_b1"]), _chunked(g["gate_W2"][0])],
                        axis=1)
    import ml_dtypes
    bf = ml_dtypes.bfloat16
    common = {
        "w1t": np.ascontiguousarray(g["gate_W1"].T).astype(bf),
        "pv": pv,
        "sc": np.array([[g["gate_b2"][0] + g["base_logit"][0]]], np.float32),
        "bt": np.ascontiguousarray(
            np.concatenate([g["fast_B"].T, g["deep_B"].T], axis=0)),
    }
    if mode == "lowrank":
        common["caug"] = np.ascontiguousarray(
            np.concatenate([g["fast_C"], g["deep_C"],
                            np.ones((D, 1), np.float32)], axis=1))
    else:
        common["ct"] = np.ascontiguousarray(
            np.concatenate([g["fast_C"].T, g["deep_C"].T], axis=0))
    if has_d:
        common["dv"] = np.ascontiguousarray(
            np.concatenate([_chunked(g["fast_D"] - g["deep_D"]),
                            _chunked(g["deep_D"])], axis=1))

    in_maps = []
    for i in range(NCORES):
        m = dict(common)
        m["kt"] = np.ascontiguousarray(
            k[i * NSH:(i + 1) * NSH, :].T).astype(bf)
        in_maps.append(m)
    return mode, has_d, in_maps


def get_nc(mode, has_d, repeat=1, sim_safe=False):
    key = (mode, has_d, repeat, sim_safe)
    if key not in _NC_CACHE:
        if mode == "fast":
            _NC_CACHE[key] = build_nc_fast(repeat, sim_safe)
        else:
            _NC_CACHE[key] = build_nc(mode, has_d, repeat, sim_safe)
    return _NC_CACHE[key]


def unscramble_fast(outF):
    """[NH*NJ*P, FH] tile-flat f16 -> [NSH, D] f32 row-major shard."""
    return (np.asarray(outF).astype(np.float32)
            .reshape(NH, NJ, P, FH).transpose(0, 3, 1, 2).reshape(NSH, D))


def assemble(results):
    out = np.empty((N, D), np.float32)
    for i in range(NCORES):
        if "outF" in results[i]:
            out[i * NSH:(i + 1) * NSH, :] = unscramble_fast(results[i]["outF"])
        else:
            out[i * NSH:(i + 1) * NSH, :] = \
                results[i]["outT"].astype(np.float32).T
    return out


def kernel(**inputs) -> np.ndarray:
    from concourse.bass_utils import run_bass_kernel_spmd

    mode, has_d, in_maps = prepare(inputs)
    nc = get_nc(mode, has_d)
    res = run_bass_kernel_spmd(nc, in_maps, core_ids=list(range(NCORES)))
    return assemble(res.results)



# revision 18
# speedup vs baseline: 1.0474x; 1.0187x over previous
"""Trainium2 Bass kernel for DualTierMiras (dual low-rank tier read + LayerNorm-gate mix).

Computes, for k [N, d]:
    v_t   = k @ (SCALE * tanh(B_t @ C_t.T) + diag(D_t)).T      (t in {fast, deep})
    h     = LayerNorm(k) * gamma + beta
    w     = sigmoid(silu(h @ W1.T + b1) @ W2.T + b2 + base_logit)
    out   = w * v_fast + (1 - w) * v_deep

Strategy: data-parallel over rows across 8 NeuronCores. All device matmuls
contract over d, so every tensor is kept in a transposed layout ([d, rows]):
the host passes k.T shards and W1.T, and the device returns out.T shards.

Two device variants:
  * "lowrank": tanh(u) ~= u whenever max|u| is provably tiny (checked on the
    host with a Cauchy-Schwarz bound, and an exact max as a second resort).
    Then k @ tanh(C B^T) == (k @ C) @ B^T up to a bounded relative error and
    the tier reads are rank-32. The gate w is folded into the tiny rank-64
    intermediate, fusing both tiers into a single K=64 matmul per out tile.
  * "tanh": materializes tanh(C B^T) per 512-column block on device and does
    the full dense tier matmuls. Used when the linearization is not safe.

All matmuls run in bf16 with fp32 PSUM accumulation.
"""

from contextlib import ExitStack

import numpy as np

N, D, R = 8192, 2048, 32
NCORES = 8
NSH = N // NCORES          # rows per core
P = 128                    # SBUF partitions
NJ = D // P                # 16 chunks of d
FH = 512                   # free-dim half of NSH (PSUM bank width in fp32)
NH = NSH // FH             # 2 halves
SCALE = 0.1
LN_EPS = 1e-5
# max |B C^T| element below which tanh(u) ~= u is used (per-element relative
# error of the tanh factor <= thr^2/3 ~= 0.33%).
LOWRANK_THR = 0.10
# fp8 fast path pre-scales: k is sent as fp8 * KS, W1*gamma as fp8 * WS
# (keeps small values out of the e4m3 subnormal range), k^2 stats * QS.
KS = 16.0
WS = 64.0
QS = 2.8284271247461903

_NC_CACHE: dict = {}


# ---------------------------------------------------------------- device build

def build_nc(mode: str, has_d: bool, repeat: int = 1, sim_safe: bool = False):
    import concourse.bacc as bacc
    import concourse.tile as tile
    from concourse import mybir

    f32 = mybir.dt.float32
    nc = bacc.Bacc("TRN2", target_bir_lowering=False, debug=False,
                   num_devices=NCORES)

    bf16 = mybir.dt.bfloat16
    kt_d = nc.dram_tensor("kt", [D, NSH], bf16, kind="ExternalInput")
    w1t_d = nc.dram_tensor("w1t", [D, D], bf16, kind="ExternalInput")
    pv_d = nc.dram_tensor("pv", [P, 64], f32, kind="ExternalInput")
    sc_d = nc.dram_tensor("sc", [1, 1], f32, kind="ExternalInput")
    bt_d = nc.dram_tensor("bt", [64, D], f32, kind="ExternalInput")
    caug_d = ct_d = dv_d = None
    if mode == "lowrank":
        caug_d = nc.dram_tensor("caug", [D, 65], f32, kind="ExternalInput")
    else:
        ct_d = nc.dram_tensor("ct", [64, D], f32, kind="ExternalInput")
    if has_d:
        dv_d = nc.dram_tensor("dv", [P, 32], f32, kind="ExternalInput")
    # bf16 output (inputs to every product term are bf16 already); the host
    # upcasts to f32 when unsharding. Halves the output DMA on the tail.
    out_d = nc.dram_tensor("outT", [D, NSH], bf16, kind="ExternalOutput")

    with tile.TileContext(nc) as tc:
        for _ in range(repeat):
            with ExitStack() as ctx:
                _emit(ctx, tc, nc, mode, has_d,
                      kt_d, w1t_d, pv_d, sc_d, bt_d, caug_d, ct_d, dv_d, out_d,
                      sim_safe=sim_safe)
    nc.compile()
    return nc


def _emit(ctx, tc, nc, mode, has_d,
          kt_d, w1t_d, pv_d, sc_d, bt_d, caug_d, ct_d, dv_d, out_d,
          sim_safe=False):
    import concourse.bass as bass  # noqa: F401
    from concourse import mybir

    f32 = mybir.dt.float32
    bf16 = mybir.dt.bfloat16
    AF = mybir.ActivationFunctionType
    ALU = mybir.AluOpType
    lowrank = mode == "lowrank"

    const = ctx.enter_context(tc.tile_pool(name="const", bufs=1))
    persist = ctx.enter_context(tc.tile_pool(name="persist", bufs=1))
    stage = ctx.enter_context(tc.tile_pool(name="stage", bufs=2))
    tmp = ctx.enter_context(tc.tile_pool(name="tmp", bufs=3))
    kt2pool = ctx.enter_context(tc.tile_pool(name="kt2p", bufs=2))
    h2pool = ctx.enter_context(tc.tile_pool(name="h2p", bufs=2))
    outpool = ctx.enter_context(tc.tile_pool(name="outp", bufs=3))
    small = ctx.enter_context(tc.tile_pool(name="small", bufs=1))
    # rotating slots for short-lived [1, FH] vectors (each costs a full
    # free-dim slot across all partitions, so don't give each a unique tag)
    svec = ctx.enter_context(tc.tile_pool(name="svec", bufs=4))

    # ---- small constants -------------------------------------------------
    pv = const.tile([P, 64], f32, tag="pv", name="pv")
    nc.sync.dma_start(pv[:], pv_d[:])
    sc = const.tile([1, 1], f32, tag="sc", name="sc")
    nc.sync.dma_start(sc[:], sc_d[:])

    if lowrank:
        # one 3D-AP DMA + one cast instead of 16 tiny strided loads, so the
        # DMA queue reaches the kt tiles sooner
        caug_v = caug_d[:].rearrange("(j p) r -> p j r", p=P)
        with tc.tile_pool(name="caugstage", bufs=1) as caugstage:
            cf3 = caugstage.tile([P, NJ, 65], f32, tag="caugf", name="caugf")
            nc.sync.dma_start(cf3[:], caug_v[:])
            cb3 = const.tile([P, NJ, 65], bf16, tag="caugbf", name="caugbf")
            nc.vector.tensor_copy(cb3[:], cf3[:])
        caug_bf = [cb3[:, j, :] for j in range(NJ)]
    ones_col = const.tile([P, 1], bf16, tag="ones", name="ones")
    nc.vector.memset(ones_col[:], 1.0)
    ones_row = const.tile([1, P], bf16, tag="onesrow", name="onesrow")
    nc.vector.memset(ones_row[:], 1.0)
    # gpsimd.partition_broadcast writes garbage on HW via this compile path;
    # broadcast [1, FH] rows across partitions with a K=1 matmul instead.
    psBC = ctx.enter_context(tc.tile_pool(name="psBC", bufs=1, space="PSUM"))

    def bcast_psum(src_row_bf16, nparts):
        pb = psBC.tile([nparts, FH], f32, tag="pbc", name="pbc")
        nc.tensor.matmul(pb[:], ones_row[0:1, 0:nparts], src_row_bf16[:],
                         start=True, stop=True)
        return pb

    w2_bf = const.tile([P, NJ], bf16, tag="w2bf", name="w2bf")
    nc.vector.tensor_copy(w2_bf[:], pv[:, 48:64])

    if has_d:
        dv = const.tile([P, 32], f32, tag="dv", name="dv")
        nc.sync.dma_start(dv[:], dv_d[:])

    # ---- load k.T, cast bf16, phase-A matmuls (stats + G) ----------------
    # kt_bf is dead after hT is built (unless a tier path needs it later), so
    # it lives in its own pool that the caller scopes appropriately.
    import os
    resident_gate = (lowrank and not has_d
                     and not os.environ.get('K_NO_RESIDENT'))
    htpool = ctx.enter_context(tc.tile_pool(name="htp", bufs=1))
    ktpool = persist
    if resident_gate:
        # resident bf16 W1 j-tiles, prefetched during the prologue
        w1pool = ctx.enter_context(tc.tile_pool(name="w1p", bufs=1))
    kt_bf = [ktpool.tile([P, NSH], bf16, tag=f"ktbf{j}", name=f"ktbf{j}") for j in range(NJ)]
    mu = [small.tile([1, FH], f32, tag=f"mu{h}", name=f"mu{h}") for h in range(NH)]
    msq = [small.tile([1, FH], f32, tag=f"msq{h}", name=f"msq{h}") for h in range(NH)]
    G_sb = None
    if lowrank:
        G_sb = [persist.tile([64, FH], bf16, tag=f"gsb{h}", name=f"gsb{h}") for h in range(NH)]

    with tc.tile_pool(name="psA", bufs=1, space="PSUM") as psA:
        if lowrank:
            psum_G = [psA.tile([65, FH], f32, tag=f"psG{h}", name=f"psG{h}") for h in range(NH)]
        else:
            psum_S = [psA.tile([1, FH], f32, tag=f"psS{h}", name=f"psS{h}") for h in range(NH)]
        psum_Q = [psA.tile([1, FH], f32, tag=f"psQ{h}", name=f"psQ{h}") for h in range(NH)]

        for j in range(NJ):
            st, sp = j == 0, j == NJ - 1
            nc.sync.dma_start(kt_bf[j][:], kt_d[j * P:(j + 1) * P, :])
            kt2 = kt2pool.tile([P, NSH], bf16, tag="kt2", name="kt2")
            nc.vector.tensor_mul(kt2[:], kt_bf[j][:], kt_bf[j][:])
            for h in range(NH):
                sl = slice(h * FH, (h + 1) * FH)
                if lowrank:
                    nc.tensor.matmul(psum_G[h][:], caug_bf[j][:],
                                     kt_bf[j][:, sl], start=st, stop=sp)
                else:
                    nc.tensor.matmul(psum_S[h][:], ones_col[:],
                                     kt_bf[j][:, sl], start=st, stop=sp)
                nc.tensor.matmul(psum_Q[h][:], ones_col[:],
                                 kt2[:, sl], start=st, stop=sp)

        w1o = None
        if resident_gate:
            # per-o column blocks via 3D AP: arrival order == the gate's
            # consumption order, so o=0 can start after 1 MB instead of 8 MB.
            w1v = w1t_d[:].rearrange("(j p) o -> p j o", p=P)
            w1o = [w1pool.tile([P, NJ, P], bf16, tag=f"w1o{o}", name=f"w1o{o}")
                   for o in range(NJ)]
            for o in range(NJ):
                nc.sync.dma_start(w1o[o][:], w1v[:, :, o * P:(o + 1) * P])

        for h in range(NH):
            if lowrank:
                nc.scalar.mul(G_sb[h][:], psum_G[h][0:64, :], SCALE)
                nc.scalar.mul(mu[h][:], psum_G[h][64:65, :], 1.0 / D)
            else:
                nc.scalar.mul(mu[h][:], psum_S[h][:], 1.0 / D)
            nc.scalar.mul(msq[h][:], psum_Q[h][:], 1.0 / D)

    # factor tiles (used only by the tier reads, so loaded after kt+W1 to
    # keep them off the critical DMA prologue): joint [64, D] for the lowrank
    # K=64 fused matmul; split per-tier [32, D] tiles in tanh mode.
    if lowrank:
        bt_bf = const.tile([64, D], bf16, tag="btbf", name="btbf")
    else:
        btt_bf = [const.tile([32, D], bf16, tag=f"btbf{t}", name=f"btbf{t}")
                  for t in range(2)]
        ctt_bf = [const.tile([32, D], bf16, tag=f"ctbf{t}", name=f"ctbf{t}")
                  for t in range(2)]
    with tc.tile_pool(name="facstage", bufs=2) as facstage:
        for q in range(2):
            qs = slice(q * (D // 2), (q + 1) * (D // 2))
            if lowrank:
                btf = facstage.tile([64, D // 2], f32, tag="btf", name="btf")
                nc.sync.dma_start(btf[:], bt_d[:, qs])
                nc.vector.tensor_copy(bt_bf[:, qs], btf[:])
            else:
                for t in range(2):
                    btf = facstage.tile([32, D // 2], f32, tag="btf", name="btf")
                    nc.sync.dma_start(btf[:], bt_d[32 * t:32 * t + 32, qs])
                    nc.vector.tensor_copy(btt_bf[t][:, qs], btf[:])
                    ctf = facstage.tile([32, D // 2], f32, tag="ctf", name="ctf")
                    nc.sync.dma_start(ctf[:], ct_d[32 * t:32 * t + 32, qs])
                    nc.vector.tensor_copy(ctt_bf[t][:, qs], ctf[:])

    # ---- LN stats finalize + broadcast -----------------------------------
    mu_b = [persist.tile([P, FH], bf16, tag=f"mub{h}", name=f"mub{h}") for h in range(NH)]
    rstd_b = [persist.tile([P, FH], bf16, tag=f"rstdb{h}", name=f"rstdb{h}") for h in range(NH)]
    for h in range(NH):
        mu2 = svec.tile([1, FH], f32, tag="sv", name="mu2")
        nc.vector.tensor_mul(mu2[:], mu[h][:], mu[h][:])
        veps = svec.tile([1, FH], f32, tag="sv", name="veps")
        # (msq + eps) - mu^2
        nc.vector.scalar_tensor_tensor(veps[:], msq[h][:], LN_EPS, mu2[:],
                                       op0=ALU.add, op1=ALU.subtract)
        rinv = svec.tile([1, FH], f32, tag="sv", name="rinv")
        nc.vector.reciprocal(rinv[:], veps[:])
        rstd_bf = svec.tile([1, FH], bf16, tag="sv", name="rstd_bf")
        nc.scalar.activation(rstd_bf[:], rinv[:], AF.Sqrt)
        mu_bf = svec.tile([1, FH], bf16, tag="sv", name="mu_bf")
        nc.scalar.copy(mu_bf[:], mu[h][:])
        nc.scalar.copy(mu_b[h][:], bcast_psum(mu_bf, P)[:])
        nc.scalar.copy(rstd_b[h][:], bcast_psum(rstd_bf, P)[:])

    # ---- gate: h = LN(k)*gamma+beta; silu(h @ W1.T + b1); logit ----------
    wv = [svec.tile([1, FH], f32, tag="wvlong", bufs=2, name=f"wv{h}")
          for h in range(NH)]
    # W1.T viewed as [p, j, o] so one DMA fetches the [2048, 128] column block
    # for a given o-chunk as a [128, 16, 128] tile (partition dim = j rows).
    w1t_v = w1t_d[:].rearrange("(j p) o -> p j o", p=P)

    def emit_ln():
        ht = [htpool.tile([P, NSH], bf16, tag=f"ht{j}", name=f"ht{j}")
              for j in range(NJ)]
        for h in range(NH):           # h-major: h=0 tiles finish first
            for j in range(NJ):
                sl = slice(h * FH, (h + 1) * FH)
                t1 = tmp.tile([P, FH], bf16, tag="lnt1", name="lnt1")
                nc.vector.tensor_sub(t1[:], kt_bf[j][:, sl], mu_b[h][:])
                t2 = tmp.tile([P, FH], bf16, tag="lnt2", name="lnt2")
                nc.vector.tensor_mul(t2[:], t1[:], rstd_b[h][:])
                nc.scalar.activation(ht[j][:, sl], t2[:], AF.Identity,
                                     bias=pv[:, 16 + j:17 + j],
                                     scale=pv[:, j:j + 1])
        return ht

    def emit_silu(s1, o, h2):
        if sim_safe:
            # CoreSim has no Silu LUT; decompose (sim-only build).
            sbt = h2pool.tile([P, FH], f32, tag="sb", name="sb")
            nc.scalar.activation(sbt[:], s1[:], AF.Identity,
                                 bias=pv[:, 32 + o:33 + o])
            sig = h2pool.tile([P, FH], f32, tag="sig", name="sig")
            nc.scalar.activation(sig[:], s1[:], AF.Sigmoid,
                                 bias=pv[:, 32 + o:33 + o])
            nc.vector.tensor_mul(h2[:], sbt[:], sig[:])
        else:
            nc.scalar.activation(h2[:], s1[:], AF.Silu,
                                 bias=pv[:, 32 + o:33 + o])

    def emit_gate_col(psB, psum_L, w1b_j_aps, o, h):
        """One (o, h) gate column: 16 accumulating matmuls + silu + logit."""
        sl = slice(h * FH, (h + 1) * FH)
        s1 = psB.tile([P, FH], f32, tag="s1", name="s1")
        for j in range(NJ):
            nc.tensor.matmul(s1[:], w1b_j_aps[j], ht[j][:, sl],
                             start=(j == 0), stop=(j == NJ - 1))
        h2 = h2pool.tile([P, FH], bf16, tag="h2", name="h2")
        emit_silu(s1, o, h2)
        nc.tensor.matmul(psum_L[h][:], w2_bf[:, o:o + 1], h2[:],
                         start=(o == 0), stop=(o == NJ - 1))

    def emit_tier_lowrank(h, psC):
        """w -> Gw -> fused K=64 tier matmul -> out DMA, for one n-half."""
        wv_bf = svec.tile([1, FH], bf16, tag="sv", name="wv_bf")
        nc.vector.tensor_copy(wv_bf[:], wv[h][:])
        nb = P if has_d else 64
        pw = bcast_psum(wv_bf, nb)
        wcat = persist.tile([64, FH], bf16, tag=f"wcat{h}", name=f"wcat{h}")
        nc.scalar.copy(wcat[0:32, :], pw[0:32, :])
        nc.scalar.activation(wcat[32:64, :], pw[32:64, :], AF.Copy,
                             bias=1.0, scale=-1.0)
        if has_d:
            wb = persist.tile([P, FH], bf16, tag=f"wb128{h}", name=f"wb128{h}")
            nc.scalar.copy(wb[:], pw[:])
        Gw = persist.tile([64, FH], bf16, tag=f"gw{h}", name=f"gw{h}")
        nc.vector.tensor_mul(Gw[:], G_sb[h][:], wcat[:])
        for m in range(NJ):
            pvt = psC.tile([P, FH], f32, tag="vt", name="vt")
            nc.tensor.matmul(pvt[:], bt_bf[0:64, m * P:(m + 1) * P],
                             Gw[:], start=True, stop=True)
            ot = outpool.tile([P, FH], bf16, tag="ot", name="ot")
            if not has_d:
                # alternate engines so the psum->sbuf copies don't pile up
                if m % 2 == 0:
                    nc.scalar.copy(ot[:], pvt[:])
                else:
                    nc.vector.tensor_copy(ot[:], pvt[:])
            else:
                sl = slice(h * FH, (h + 1) * FH)
                dmix = tmp.tile([P, FH], bf16, tag="dmix", name="dmix")
                nc.vector.tensor_scalar(dmix[:], wb[:],
                                        dv[:, m:m + 1], dv[:, 16 + m:17 + m],
                                        op0=ALU.mult, op1=ALU.add)
                c = tmp.tile([P, FH], f32, tag="dc", name="dc")
                nc.vector.tensor_mul(c[:], kt_bf[m][:, sl], dmix[:])
                nc.vector.tensor_add(ot[:], pvt[:], c[:])
            nc.sync.dma_start(
                out_d[m * P:(m + 1) * P, h * FH:(h + 1) * FH], ot[:])

    if resident_gate:
        # Resident bf16 W1: one DMA+cast pass, reused by both n-halves, so
        # the gate runs h-outer and half 0's tier-read/output tail overlaps
        # half 1's gate matmuls.  kt_bf's pool closes once hT exists.
        ht = emit_ln()
        with ExitStack() as gctx:
            psC = gctx.enter_context(tc.tile_pool(name="psC", bufs=2,
                                                  space="PSUM"))
            with tc.tile_pool(name="psB", bufs=2, space="PSUM") as psB, \
                 tc.tile_pool(name="psL", bufs=1, space="PSUM") as psL:
                psum_L = [psL.tile([1, FH], f32, tag=f"psL{h}",
                                   name=f"psL{h}") for h in range(NH)]
                interleave = not os.environ.get('K_NO_INTERLEAVE')
                for h in range(NH):
                    for o in range(NJ):
                        aps = [w1o[o][:, j, :] for j in range(NJ)]
                        emit_gate_col(psB, psum_L, aps, o, h)
                    nc.scalar.activation(wv[h][:], psum_L[h][:], AF.Sigmoid,
                                         bias=sc[0:1, 0:1])
                    if interleave:
                        emit_tier_lowrank(h, psC)
                if not interleave:
                    for h in range(NH):
                        emit_tier_lowrank(h, psC)
    else:
        ht = emit_ln()
        with ExitStack() as gctx:
            w1bp = gctx.enter_context(tc.tile_pool(name="w1bp", bufs=2))
            with tc.tile_pool(name="psB", bufs=2, space="PSUM") as psB, \
                 tc.tile_pool(name="psL", bufs=1, space="PSUM") as psL:
                psum_L = [psL.tile([1, FH], f32, tag=f"psL{h}",
                                   name=f"psL{h}") for h in range(NH)]
                for o in range(NJ):
                    w1b = w1bp.tile([P, NJ, P], bf16, tag="w1b", name="w1b")
                    nc.sync.dma_start(w1b[:], w1t_v[:, :, o * P:(o + 1) * P])
                    for h in range(NH):
                        aps = [w1b[:, j, :] for j in range(NJ)]
                        emit_gate_col(psB, psum_L, aps, o, h)
                for h in range(NH):
                    nc.scalar.activation(wv[h][:], psum_L[h][:], AF.Sigmoid,
                                         bias=sc[0:1, 0:1])

    # ---- tier reads + mix ------------------------------------------------
    if lowrank:
        if not resident_gate:
            with tc.tile_pool(name="psC", bufs=3, space="PSUM") as psC:
                for h in range(NH):
                    emit_tier_lowrank(h, psC)
    else:
        # Full path: materialize M_t = tanh(C_t B_t^T) per 512-col block.
        wpb = [persist.tile([P, FH], f32, tag=f"wpb{h}", name=f"wpb{h}") for h in range(NH)]
        wqb = [persist.tile([P, FH], f32, tag=f"wqb{h}", name=f"wqb{h}") for h in range(NH)]
        wb128 = []
        for h in range(NH):
            wv_bf = svec.tile([1, FH], bf16, tag="sv", name="wv_bf")
            nc.vector.tensor_copy(wv_bf[:], wv[h][:])
            pw = bcast_psum(wv_bf, P)
            # wpb = SCALE*w, wqb = SCALE*(1-w), folded into the psum copies
            nc.scalar.mul(wpb[h][:], pw[:], SCALE)
            nc.scalar.activation(wqb[h][:], pw[:], AF.Copy,
                                 bias=SCALE, scale=-SCALE)
            if has_d:
                wb = persist.tile([P, FH], bf16, tag=f"wb128{h}", name=f"wb128{h}")
                nc.scalar.copy(wb[:], pw[:])
                wb128.append(wb)

        with ExitStack() as tctx:
            mpool = tctx.enter_context(tc.tile_pool(name="mtiles", bufs=1))
            psD = tctx.enter_context(tc.tile_pool(name="psD", bufs=2,
                                                  space="PSUM"))
            for mg in range(D // FH):
                mt = [[], []]
                for t in range(2):
                    for j in range(NJ):
                        pm = psD.tile([P, FH], f32, tag="pm", name="pm",
                                      bufs=1)
                        nc.tensor.matmul(
                            pm[:], ctt_bf[t][:, j * P:(j + 1) * P],
                            btt_bf[t][:, mg * FH:(mg + 1) * FH],
                            start=True, stop=True)
                        mtile = mpool.tile([P, FH], bf16, tag=f"m{t}_{j}", name=f"m{t}_{j}")
                        nc.scalar.activation(mtile[:], pm[:], AF.Tanh)
                        mt[t].append(mtile)
                for s in range(FH // P):
                    m = mg * (FH // P) + s
                    for h in range(NH):
                        sl = slice(h * FH, (h + 1) * FH)
                        pf = psD.tile([P, FH], f32, tag="pf", name="pf")
                        for j in range(NJ):
                            nc.tensor.matmul(pf[:],
                                             mt[0][j][:, s * P:(s + 1) * P],
                                             kt_bf[j][:, sl],
                                             start=(j == 0), stop=(j == NJ - 1))
                        pd_ = psD.tile([P, FH], f32, tag="pd", name="pd")
                        for j in range(NJ):
                            nc.tensor.matmul(pd_[:],
                                             mt[1][j][:, s * P:(s + 1) * P],
                                             kt_bf[j][:, sl],
                                             start=(j == 0), stop=(j == NJ - 1))
                        t0 = tmp.tile([P, FH], f32, tag="t0", name="t0")
                        nc.vector.tensor_mul(t0[:], pf[:], wpb[h][:])
                        t1 = tmp.tile([P, FH], f32, tag="t1", name="t1")
                        nc.vector.tensor_mul(t1[:], pd_[:], wqb[h][:])
                        ot = outpool.tile([P, FH], bf16, tag="ot", name="ot")
                        nc.vector.tensor_add(ot[:], t0[:], t1[:])
                        if has_d:
                            dmix = tmp.tile([P, FH], bf16, tag="dmix", name="dmix")
                            nc.vector.tensor_scalar(dmix[:], wb128[h][:],
                                                    dv[:, m:m + 1],
                                                    dv[:, 16 + m:17 + m],
                                                    op0=ALU.mult, op1=ALU.add)
                            c = tmp.tile([P, FH], f32, tag="dc", name="dc")
                            nc.vector.tensor_mul(c[:], kt_bf[m][:, sl], dmix[:])
                            ot2 = outpool.tile([P, FH], bf16, tag="ot2",
                                               name="ot2")
                            nc.vector.tensor_add(ot2[:], ot[:], c[:])
                            ot = ot2
                        nc.sync.dma_start(
                            out_d[m * P:(m + 1) * P, h * FH:(h + 1) * FH],
                            ot[:])


# ------------------------------------------------------- fast path (fp8 gate)

def build_nc_fast(repeat: int = 1, sim_safe: bool = False):
    """Lowrank/no-D fast path: LN folded into W1, fp8 DoubleRow gate matmul.

    Per (o, h) gate column the PSUM group is 8 fp8-DR matmuls (K=256 each)
    plus one K=1 fp16 matmul adding the rank-1 LN mean correction
    (-mu_r) * (KS*WS * W1@gamma)_o; z = psum * (rstd/(KS*WS)) then
    h2 = silu(z + (W1@beta + b1)).
    """
    import concourse.bacc as bacc
    import concourse.tile as tile
    from concourse import mybir

    f32 = mybir.dt.float32
    f16 = mybir.dt.float16
    f8 = mybir.dt.float8e4
    nc = bacc.Bacc("TRN2", target_bir_lowering=False, debug=False,
                   num_devices=NCORES)

    kt16_d = nc.dram_tensor("kt16", [D, NSH], f16, kind="ExternalInput")
    kt8_d = nc.dram_tensor("kt8", [D, NSH], f8, kind="ExternalInput")
    w1p8_d = nc.dram_tensor("w1p8", [D, D], f8, kind="ExternalInput")
    caug_d = nc.dram_tensor("caug2", [P, NJ * 65], f16, kind="ExternalInput")
    bt_d = nc.dram_tensor("bt2", [64, D], f16, kind="ExternalInput")
    c1s_d = nc.dram_tensor("c1s", [1, D], f16, kind="ExternalInput")
    cb_d = nc.dram_tensor("cb", [P, NJ], f32, kind="ExternalInput")
    w2b_d = nc.dram_tensor("w2b", [P, NJ], f16, kind="ExternalInput")
    sc_d = nc.dram_tensor("sc", [1, 1], f32, kind="ExternalInput")
    out_d = nc.dram_tensor("outF", [NH * NJ * P, FH], f16,
                           kind="ExternalOutput")

    with tile.TileContext(nc) as tc:
        for _ in range(repeat):
            with ExitStack() as ctx:
                _emit_fast(ctx, tc, nc, kt16_d, kt8_d, w1p8_d, caug_d, bt_d,
                           c1s_d, cb_d, w2b_d, sc_d, out_d, sim_safe=sim_safe)
    nc.compile()
    return nc


def _emit_fast(ctx, tc, nc, kt16_d, kt8_d, w1p8_d, caug_d, bt_d,
               c1s_d, cb_d, w2b_d, sc_d, out_d, sim_safe=False):
    from concourse import mybir

    f32 = mybir.dt.float32
    f16 = mybir.dt.float16
    f8 = mybir.dt.float8e4
    AF = mybir.ActivationFunctionType
    ALU = mybir.AluOpType
    DR = mybir.MatmulPerfMode.DoubleRow

    const = ctx.enter_context(tc.tile_pool(name="const", bufs=1))
    persist = ctx.enter_context(tc.tile_pool(name="persist", bufs=1))
    tmp = ctx.enter_context(tc.tile_pool(name="tmp", bufs=3))
    h2pool = ctx.enter_context(tc.tile_pool(name="h2p", bufs=2))
    outpool = ctx.enter_context(tc.tile_pool(name="outp", bufs=4))
    small = ctx.enter_context(tc.tile_pool(name="small", bufs=1))
    svec = ctx.enter_context(tc.tile_pool(name="svec", bufs=4))

    # ---- input DMAs (order = criticality) --------------------------------
    # caug first (G stats need it with kt16[0]), then the kt16 stream for
    # phase A, then kt8 + first W1 blocks (gate prerequisites), then the tiny
    # tensors (consumed mid-gate at the earliest), then the rest of W1.
    caug_sb = const.tile([P, NJ * 65], f16, tag="caug", name="caug")
    nc.sync.dma_start(caug_sb[:], caug_d[:])

    ones_row = const.tile([1, P], f16, tag="onesrow", name="onesrow")
    nc.vector.memset(ones_row[:], 1.0)
    ones8 = const.tile([P, 2, 16], f8, tag="ones8", name="ones8")
    nc.vector.memset(ones8[:], 1.0)

    kt16 = [persist.tile([P, NSH], f16, tag=f"kt16_{j}", name=f"kt16_{j}")
            for j in range(NJ)]
    for j in range(NJ):
        nc.sync.dma_start(kt16[j][:], kt16_d[j * P:(j + 1) * P, :])

    kt8 = persist.tile([P, NJ, NSH], f8, tag="kt8", name="kt8")
    kt8_v = kt8_d[:].rearrange("(j p) n -> p j n", p=P)
    HJ = NJ // 2
    nc.sync.dma_start(kt8[:, 0:HJ, :], kt8_v[:, 0:HJ, :])
    nc.sync.dma_start(kt8[:, HJ:NJ, :], kt8_v[:, HJ:NJ, :])

    w1o = [persist.tile([P, NJ, P], f8, tag=f"w1o{o}", name=f"w1o{o}")
           for o in range(NJ)]

    def dma_w1(o):
        nc.sync.dma_start(w1o[o][:], w1p8_d[o * P:(o + 1) * P, :]
                          .rearrange("p (j m) -> p j m", j=NJ))

    dma_w1(0)
    dma_w1(1)
    c1s_sb = const.tile([1, D], f16, tag="c1s", name="c1s")
    nc.sync.dma_start(c1s_sb[:], c1s_d[:])
    cb_sb = const.tile([P, NJ], f32, tag="cb", name="cb")
    nc.sync.dma_start(cb_sb[:], cb_d[:])
    w2b_sb = const.tile([P, NJ], f16, tag="w2b", name="w2b")
    nc.sync.dma_start(w2b_sb[:], w2b_d[:])
    sc = const.tile([1, 1], f32, tag="sc", name="sc")
    nc.sync.dma_start(sc[:], sc_d[:])
    for o in range(2, NJ):
        dma_w1(o)
    bt_sb = const.tile([64, D], f16, tag="bt", name="bt")
    nc.sync.dma_start(bt_sb[:], bt_d[:])

    # ---- phase A: stats --------------------------------------------------
    kt28 = persist.tile([P, NJ, NSH], f8, tag="kt28", name="kt28")
    G_sb = [persist.tile([64, FH], f16, tag=f"gsb{h}", name=f"gsb{h}")
            for h in range(NH)]
    negmu = [small.tile([1, FH], f16, tag=f"negmu{h}", name=f"negmu{h}")
             for h in range(NH)]
    rstd_b = [persist.tile([P, FH], f16, tag=f"rstdb{h}", name=f"rstdb{h}")
              for h in range(NH)]
    psBC = ctx.enter_context(tc.tile_pool(name="psBC", bufs=1, space="PSUM"))

    with tc.tile_pool(name="psA", bufs=1, space="PSUM") as psA:
        psum_G = [psA.tile([65, FH], f32, tag=f"psG{h}", name=f"psG{h}")
                  for h in range(NH)]
        psum_Q = [psA.tile([1, FH], f32, tag=f"psQ{h}", name=f"psQ{h}")
                  for h in range(NH)]
        sqs = QS ** 0.5
        for j in range(NJ):
            st, sp = j == 0, j == NJ - 1
            # split the squares across Act/DVE so neither paces phase A
            # (Square is present in every activation table: no table load)
            if j % 2 == 0:
                nc.scalar.activation(kt28[:, j, :], kt16[j][:], AF.Square,
                                     scale=sqs)
            else:
                nc.vector.scalar_tensor_tensor(
                    kt28[:, j, :], kt16[j][:], QS, kt16[j][:],
                    op0=ALU.mult, op1=ALU.mult)
            for h in range(NH):
                sl = slice(h * FH, (h + 1) * FH)
                nc.tensor.matmul(psum_G[h][:],
                                 caug_sb[:, j * 65:(j + 1) * 65],
                                 kt16[j][:, sl], start=st, stop=sp)
            if j % 2 == 1:
                jp = j // 2
                for h in range(NH):
                    sl = slice(h * FH, (h + 1) * FH)
                    nc.tensor.matmul(psum_Q[h][:], ones8[:, :, 0:1],
                                     kt28[:, 2 * jp:2 * jp + 2, sl],
                                     start=(jp == 0), stop=(jp == NJ // 2 - 1),
                                     perf_mode=DR)

        for h in range(NH):
            nc.scalar.mul(G_sb[h][:], psum_G[h][0:64, :], SCALE)
            nc.scalar.mul(negmu[h][:], psum_G[h][64:65, :], -1.0 / D)
            msq = svec.tile([1, FH], f32, tag="sv", name="msq")
            nc.scalar.mul(msq[:], psum_Q[h][:], 1.0 / (QS * D))
            mu2 = svec.tile([1, FH], f32, tag="sv", name="mu2")
            nc.vector.tensor_mul(mu2[:], negmu[h][:], negmu[h][:])
            veps = svec.tile([1, FH], f32, tag="sv", name="veps")
            nc.vector.scalar_tensor_tensor(veps[:], msq[:], LN_EPS, mu2[:],
                                           op0=ALU.add, op1=ALU.subtract)
            rinv = svec.tile([1, FH], f32, tag="sv", name="rinv")
            nc.vector.reciprocal(rinv[:], veps[:])
            rstd_row = svec.tile([1, FH], f16, tag="sv", name="rstd_row")
            # sqrt(rinv / (KS*WS)^2) = rstd / (KS*WS)
            nc.scalar.activation(rstd_row[:], rinv[:], AF.Sqrt,
                                 scale=1.0 / (KS * WS) ** 2)
            pb = psBC.tile([P, FH], f32, tag="pbc", name="pbc")
            nc.tensor.matmul(pb[:], ones_row[0:1, 0:P], rstd_row[:],
                             start=True, stop=True)
            nc.scalar.copy(rstd_b[h][:], pb[:])

    # ---- gate + tier, per n-half ----------------------------------------
    def emit_silu(t, o, h2):
        if sim_safe:
            sbt = h2pool.tile([P, FH], f32, tag="sb", name="sb")
            nc.scalar.activation(sbt[:], t[:], AF.Identity,
                                 bias=cb_sb[:, o:o + 1])
            sig = h2pool.tile([P, FH], f32, tag="sig", name="sig")
            nc.scalar.activation(sig[:], t[:], AF.Sigmoid,
                                 bias=cb_sb[:, o:o + 1])
            nc.vector.tensor_mul(h2[:], sbt[:], sig[:])
        else:
            nc.scalar.activation(h2[:], t[:], AF.Silu,
                                 bias=cb_sb[:, o:o + 1])

    psC = ctx.enter_context(tc.tile_pool(name="psC", bufs=3, space="PSUM"))
    with tc.tile_pool(name="psB", bufs=3, space="PSUM") as psB, \
         tc.tile_pool(name="psL", bufs=1, space="PSUM") as psL:
        for h in range(NH):
            sl = slice(h * FH, (h + 1) * FH)
            psum_L = psL.tile([1, FH], f32, tag="psL", name=f"psL{h}")
            for o in range(NJ):
                s1 = psB.tile([P, FH], f32, tag="s1", name="s1")
                for jp in range(NJ // 2):
                    nc.tensor.matmul(s1[:], w1o[o][:, 2 * jp:2 * jp + 2, :],
                                     kt8[:, 2 * jp:2 * jp + 2, sl],
                                     start=(jp == 0), stop=False,
                                     perf_mode=DR)
                nc.tensor.matmul(s1[:], c1s_sb[0:1, o * P:(o + 1) * P],
                                 negmu[h][:], start=False, stop=True)
                t = tmp.tile([P, FH], f16, tag="t", name="t")
                nc.vector.tensor_mul(t[:], s1[:], rstd_b[h][:])
                h2 = h2pool.tile([P, FH], f16, tag="h2", name="h2")
                emit_silu(t, o, h2)
                nc.tensor.matmul(psum_L[:], w2b_sb[:, o:o + 1], h2[:],
                                 start=(o == 0), stop=(o == NJ - 1))
            # w = sigmoid(logit+2*sch) = 0.5 + 0.5*tanh(0.5*logit + sch);
            # Tanh lives in the same activation table as Silu/Copy, so the
            # gate->tier transition needs no table swap (Sigmoid would).
            wv16 = svec.tile([1, FH], f16, tag="sv", name="wv16")
            nc.scalar.activation(wv16[:], psum_L[:], AF.Tanh,
                                 bias=sc[0:1, 0:1], scale=0.5)
            pw = psBC.tile([P, FH], f32, tag="pbc", name="pw")
            nc.tensor.matmul(pw[0:64, :], ones_row[0:1, 0:64], wv16[:],
                             start=True, stop=True)
            wcat = persist.tile([64, FH], f16, tag=f"wcat{h}",
                                name=f"wcat{h}")
            nc.scalar.activation(wcat[0:32, :], pw[0:32, :], AF.Copy,
                                 bias=0.5, scale=0.5)
            nc.scalar.activation(wcat[32:64, :], pw[32:64, :], AF.Copy,
                                 bias=0.5, scale=-0.5)
            Gw = persist.tile([64, FH], f16, tag=f"gw{h}", name=f"gw{h}")
            nc.vector.tensor_mul(Gw[:], G_sb[h][:], wcat[:])
            for m in range(NJ):
                pvt = psC.tile([P, FH], f32, tag="vt", name="vt")
                nc.tensor.matmul(pvt[:], bt_sb[0:64, m * P:(m + 1) * P],
                                 Gw[:], start=True, stop=True)
                ot = outpool.tile([P, FH], f16, tag="ot", name="ot")
                # GPSIMD cannot read PSUM (BIR verifier), so alternate the
                # psum->sbuf copies between the Act and DVE engines
                if m % 2 == 0:
                    nc.scalar.copy(ot[:], pvt[:])
                else:
                    nc.vector.tensor_copy(ot[:], pvt[:])
                nc.sync.dma_start(
                    out_d[(h * NJ + m) * P:(h * NJ + m + 1) * P, :], ot[:])


# ---------------------------------------------------------------- host side

def _chunked(vec):
    """[2048] -> [128, 16]; column j holds elements j*128 .. j*128+127."""
    return np.ascontiguousarray(np.asarray(vec, np.float32).reshape(NJ, P).T)


def _pick_mode(fast_B, fast_C, deep_B, deep_C):
    """lowrank iff max |B C^T| provably <= LOWRANK_THR."""
    worst = 0.0
    for B, C in ((fast_B, fast_C), (deep_B, deep_C)):
        bound = (np.linalg.norm(B, axis=1).max() *
                 np.linalg.norm(C, axis=1).max())
        if bound > LOWRANK_THR:
            bound = float(np.abs(B @ C.T).max())
        worst = max(worst, float(bound))
    return "lowrank" if worst <= LOWRANK_THR else "tanh"


def prepare_fast(g):
    """in_maps for the fp8 fast path (lowrank, no diag-D)."""
    import ml_dtypes
    f8 = ml_dtypes.float8_e4m3
    f16 = np.float16
    k = g["k"]
    W1g = g["gate_W1"] * g["ln_gamma"][None, :]
    w1p8 = np.ascontiguousarray(
        (W1g * WS).reshape(NJ, P, NJ, P).transpose(0, 3, 2, 1)
        .reshape(D, D)).astype(f8)
    caug = np.concatenate([g["fast_C"], g["deep_C"],
                           np.ones((D, 1), np.float32)], axis=1)
    caug2 = np.ascontiguousarray(
        caug.reshape(NJ, P, 65).transpose(1, 0, 2).reshape(P, NJ * 65)
    ).astype(f16)
    common = {
        "w1p8": w1p8,
        "caug2": caug2,
        "bt2": np.ascontiguousarray(
            np.concatenate([g["fast_B"].T, g["deep_B"].T], axis=0)
        ).astype(f16),
        "c1s": ((g["gate_W1"] @ g["ln_gamma"]) * (KS * WS)
                ).astype(f16).reshape(1, D),
        "cb": _chunked(g["gate_W1"] @ g["ln_beta"] + g["gate_b1"]),
        "w2b": _chunked(g["gate_W2"][0]).astype(f16),
        # tanh-form gate: w = 0.5 + 0.5*tanh(0.5*logit + sc), sc = (b2+base)/2
        "sc": np.array([[(g["gate_b2"][0] + g["base_logit"][0]) / 2]],
                       np.float32),
    }
    in_maps = []
    for i in range(NCORES):
        m = dict(common)
        ktT = np.ascontiguousarray(k[i * NSH:(i + 1) * NSH, :].T)
        m["kt16"] = ktT.astype(f16)
        m["kt8"] = (ktT * KS).astype(f8)
        in_maps.append(m)
    return in_maps


def prepare(inputs):
    """-> (mode, has_d, in_maps) for the 8 cores."""
    g = {k: np.asarray(v, np.float32) for k, v in inputs.items()}
    k = g["k"]
    assert k.shape == (N, D), k.shape

    mode = _pick_mode(g["fast_B"], g["fast_C"], g["deep_B"], g["deep_C"])
    has_d = bool(np.any(g["fast_D"]) or np.any(g["deep_D"]))
    if mode == "lowrank" and not has_d:
        return "fast", False, prepare_fast(g)

    pv = np.concatenate([_chunked(g["ln_gamma"]), _chunked(g["ln_beta"]),
                         _chunked(g["gate_b1"]), _chunked(g["gate_W2"][0])],
                        axis=1)
    import ml_dtypes
    bf = ml_dtypes.bfloat16
    common = {
        "w1t": np.ascontiguousarray(g["gate_W1"].T).astype(bf),
        "pv": pv,
        "sc": np.array([[g["gate_b2"][0] + g["base_logit"][0]]], np.float32),
        "bt": np.ascontiguousarray(
            np.concatenate([g["fast_B"].T, g["deep_B"].T], axis=0)),
    }
    if mode == "lowrank":
        common["caug"] = np.ascontiguousarray(
            np.concatenate([g["fast_C"], g["deep_C"],
                            np.ones((D, 1), np.float32)], axis=1))
    else:
        common["ct"] = np.ascontiguousarray(
            np.concatenate([g["fast_C"].T, g["deep_C"].T], axis=0))
    if has_d:
        common["dv"] = np.ascontiguousarray(
            np.concatenate([_chunked(g["fast_D"] - g["deep_D"]),
                            _chunked(g["deep_D"])], axis=1))

    in_maps = []
    for i in range(NCORES):
        m = dict(common)
        m["kt"] = np.ascontiguousarray(
            k[i * NSH:(i + 1) * NSH, :].T).astype(bf)
        in_maps.append(m)
    return mode, has_d, in_maps


def get_nc(mode, has_d, repeat=1, sim_safe=False):
    key = (mode, has_d, repeat, sim_safe)
    if key not in _NC_CACHE:
        if mode == "fast":
            _NC_CACHE[key] = build_nc_fast(repeat, sim_safe)
        else:
            _NC_CACHE[key] = build_nc(mode, has_d, repeat, sim_safe)
    return _NC_CACHE[key]


def unscramble_fast(outF):
    """[NH*NJ*P, FH] tile-flat f16 -> [NSH, D] f32 row-major shard."""
    return (np.asarray(outF).astype(np.float32)
            .reshape(NH, NJ, P, FH).transpose(0, 3, 1, 2).reshape(NSH, D))


def assemble(results):
    out = np.empty((N, D), np.float32)
    for i in range(NCORES):
        if "outF" in results[i]:
            out[i * NSH:(i + 1) * NSH, :] = unscramble_fast(results[i]["outF"])
        else:
            out[i * NSH:(i + 1) * NSH, :] = \
                results[i]["outT"].astype(np.float32).T
    return out


def kernel(**inputs) -> np.ndarray:
    from concourse.bass_utils import run_bass_kernel_spmd

    mode, has_d, in_maps = prepare(inputs)
    nc = get_nc(mode, has_d)
    res = run_bass_kernel_spmd(nc, in_maps, core_ids=list(range(NCORES)))
    return assemble(res.results)

